# revision 26
# baseline (speedup 1.0000x reference)
"""Trainium2 Bass kernel for CustomAttention (ViT-style windowed attention).

Math (per batch element):
  qkv = x @ qkv_w.T + qkv_b            -> q, k, v  [H=12 heads, D=64]
  s   = (q * D^-0.5) @ k.T             masked by a fixed 24x24-grid window
  attn = softmax(s)                    (CLS row/col always attended)
  out  = attn @ v                      -> concat heads -> @ proj_w.T + proj_b

Sharding: data-parallel over batch across 8 cores (4 images/core).

Key device-side choices:
  - The window mask in row-major token order is a band: patch key j is
    attended only by queries in [j-25, j+25] (plus CLS row/col). Scores and
    attn@v therefore run BANDED per 128-key tile: each key tile streams only
    its ~180-column query window instead of all 578. Key tile 0 keeps the
    full query range (its row 0 is the CLS key, attended by every query).
    The CLS query (attends all keys) lands in column 0 of each window via
    N=1 matmuls; its attn@v contributions accumulate into output column 0.
  - Window score tiles are packed pairwise into one PSUM bank (kt1+kt2,
    kt3+kt4) so exp and mask run once per packed tile.
  - attn@v accumulates banded segments into one [65, 580] PSUM tile spanning
    2 banks; matmuls split at the 512-column bank boundary and the first
    writer of each bank uses start=True (clears has_written for the bank).
  - All matmul operands are bf16 (fp32 PSUM accumulation): bf16 streams
    1 col/cycle at any N (fp32r needs N>=256), enables fast weight load, and
    halves DMA/SBUF. Measured max-rel error ~4e-3 vs the 2e-2 gate.
  - softmax runs unnormalized (no max subtraction; |s| small): exp then
    mask-multiply. v carries an interleaved ones column so attn@v also
    yields the softmax denominators (row 64). Head outputs + denominators
    stage through one bf16 copy; normalization is deferred one pair:
    Scalar-engine reciprocal, DRAM-source partition-broadcast DMA, bf16
    multiply before proj.
  - a burst of junk warmup matmuls at kernel start keeps the PE HAM clock
    gate at full rate while x/weights stream in (x DMAs issued first).
"""

import ml_dtypes
import numpy as np

import concourse.bass as bass
import concourse.mybir as mybir
from concourse import bacc
from concourse.bass_utils import run_bass_kernel_spmd
from concourse.tile import TileContext

B, N, C = 32, 577, 768
H, D = 12, 64
NCORES = 8
BPC = B // NCORES            # batches per core
NP = N + 1                   # padded token count (even)
TP = BPC * NP
T = BPC * N
SCALE = float(D) ** -0.5     # 0.125, exact in bf16
F32 = mybir.dt.float32
BF16 = mybir.dt.bfloat16
P = 128

CT = C // P                                      # 6 contraction tiles
KT = [(0, 128), (128, 128), (256, 128), (384, 128), (512, 65)]
QCH = [(0, 290), (290, 288)]                     # qk / kt0-score chunks
VCH = [(0, 512), (512, 256)]                     # v / proj output chunks
# banded windows for key tiles 1..4: (grp, off, a, blen, k0, ksz)
# grp selects the packed psum/es tile (0: kt1+kt2, 1: kt3+kt4); within it,
# cols off+0/off+1 = scores vs q tokens 0 (CLS) and 1 (masked to zero),
# cols off+2..off+1+blen = q tokens [a, a+blen). All col offsets/sizes even.
WIN = [
    (0, 0, 102, 180, 128, 128),
    (0, 182, 230, 180, 256, 128),
    (1, 0, 358, 180, 384, 128),
    (1, 182, 486, 92, 512, 65),
]
# key tile 0's scores vs q tokens [512, 578) also pack into the grp-0 tile
# at cols [K0B, K0B+66) (its [0, 512) chunk fills a whole bank on its own).
K0B = 364
GW = [430, 276]              # packed window tile widths
MW = sum(GW)                 # banded mask tile width
AF = mybir.ActivationFunctionType
ALU = mybir.AluOpType


def _build_mask_np():
    img = 24
    p = np.arange(img * img)
    pi, pj = p // img, p % img
    ok = (np.abs(pi[:, None] - pi[None, :]) <= 1) & (
        np.abs(pj[:, None] - pj[None, :]) <= 1
    )
    m = np.zeros((N, N), dtype=np.float32)
    m[1:, 1:] = ok
    m[0, :] = True
    m[:, 0] = True
    return m


def _bcast_ap(ap1d, parts):
    """1-row AP -> [parts, n] with partition stride 0 (DRAM-source DMA)."""
    return bass.AP(
        tensor=ap1d.tensor, offset=ap1d.offset, ap=[[0, parts]] + list(ap1d.ap)[-1:]
    )


def _build_program():
    nc = bacc.Bacc("TRN2", target_bir_lowering=False, debug=False)
    xT = nc.dram_tensor("xT", [C, TP], BF16, kind="ExternalInput").ap()
    wqkT = nc.dram_tensor("wqkT", [C, 2 * C], BF16, kind="ExternalInput").ap()
    wvT = nc.dram_tensor("wvT", [C, C], BF16, kind="ExternalInput").ap()
    wpT = nc.dram_tensor("wpT", [C, C], BF16, kind="ExternalInput").ap()
    bqk = nc.dram_tensor("bqk", [2 * C], F32, kind="ExternalInput").ap()
    bv = nc.dram_tensor("bv", [C], F32, kind="ExternalInput").ap()
    bp = nc.dram_tensor("bp", [C], F32, kind="ExternalInput").ap()
    mask0d = nc.dram_tensor("mask0d", [P, NP], BF16, kind="ExternalInput").ap()
    maskwd = nc.dram_tensor("maskwd", [P, MW], BF16, kind="ExternalInput").ap()
    ones12 = nc.dram_tensor("ones12", [H], BF16, kind="ExternalInput").ap()
    y = nc.dram_tensor("y", [T, C], F32, kind="ExternalOutput").ap()

    with TileContext(nc) as tc:
        with (
            tc.tile_pool(name="singles", bufs=1) as singles,
            tc.tile_pool(name="xp", bufs=2) as xp,
            tc.tile_pool(name="qkp", bufs=2) as qkp,
            tc.tile_pool(name="vtp", bufs=2) as vtp,
            tc.tile_pool(name="ocp", bufs=2) as ocp,
            tc.tile_pool(name="esp", bufs=2) as esp,
            tc.tile_pool(name="esw", bufs=4) as eswp,
            tc.tile_pool(name="rcp", bufs=3) as rcpp,
            tc.tile_pool(name="ysp", bufs=2) as ysp,
            tc.tile_pool(name="pmm", bufs=3, space="PSUM") as pmm,
            tc.tile_pool(name="psc", bufs=3, space="PSUM") as psc,
            tc.tile_pool(name="poe", bufs=1, space="PSUM") as poe,
            tc.tile_pool(name="drp", bufs=4, space="DRAM") as drp,
        ):
            # ---- prefetch batch 0's x before the weights ----
            def emit_x_dma(b):
                xT_b = []
                for ct in range(CT):
                    t = xp.tile([P, NP], BF16, tag=f"x{ct}", name=f"x{ct}")
                    nc.sync.dma_start(
                        t[:], xT[ct * P : (ct + 1) * P, b * NP : (b + 1) * NP]
                    )
                    xT_b.append(t)
                return xT_b

            xT_b = emit_x_dma(0)

            # ---- PE warmup: junk matmuls overlap the input DMAs and get
            # the HAM clock gate to 8/8 before real work starts ----
            wup = singles.tile([P, 512], BF16, tag="wup")
            nc.vector.memset(wup[:], 1.0)
            for i in range(24):
                ps = pmm.tile([P, 512], F32, tag="pb", name="ps")
                nc.tensor.matmul(ps[:, :512], wup[:, 0:P], wup[:, 0:512],
                                 start=True, stop=True, skip_group_check=True)

            # ---- persistent loads (v/qk weights first) ----
            wv_sb = []
            wqk_sb = []
            wp_sb = []
            for ct in range(CT):
                t = singles.tile([P, C], BF16, tag=f"wv{ct}")
                nc.sync.dma_start(t[:], wvT[ct * P : (ct + 1) * P, :])
                wv_sb.append(t)
            for ct in range(CT):
                t = singles.tile([P, 2 * C], BF16, tag=f"wqk{ct}")
                wqk_sb.append(t)
            # stream wqk in head-pair order so qk(0) starts before the rest
            # of the weights arrive
            for hp in range(H // 2):
                for ft in (hp, CT + hp):
                    for ct in range(CT):
                        nc.sync.dma_start(
                            wqk_sb[ct][:, ft * P : (ft + 1) * P],
                            wqkT[ct * P : (ct + 1) * P, ft * P : (ft + 1) * P],
                        )
            bqk_sb = singles.tile([P, 2 * C // P], F32, tag="bqk")
            nc.sync.dma_start(bqk_sb[:], bqk.rearrange("(o p) -> p o", p=P))
            bv_sb = singles.tile([P, C], F32, tag="bv")
            nc.sync.dma_start(bv_sb[:], _bcast_ap(bv, P))
            ones_sb = singles.tile([P, H], BF16, tag="ones_sb")
            nc.sync.dma_start(ones_sb[:], _bcast_ap(ones12, P))
            mask0_sb = singles.tile([P, NP], BF16, tag="mask0")
            nc.sync.dma_start(mask0_sb[:], mask0d[:, :])
            maskw_sb = singles.tile([P, MW], BF16, tag="maskw")
            nc.sync.dma_start(maskw_sb[:], maskwd[:, :])
            for ct in range(CT):
                t = singles.tile([P, C], BF16, tag=f"wp{ct}")
                nc.sync.dma_start(t[:], wpT[ct * P : (ct + 1) * P, :])
                wp_sb.append(t)
            bp_sb = singles.tile([P, C], F32, tag="bp")
            nc.sync.dma_start(bp_sb[:], _bcast_ap(bp, P))

            def emit_v(xT_b):
                v_tok = []
                for mt, (m0, msz) in enumerate(KT):
                    vt = vtp.tile([P, H, D + 1], BF16, tag=f"vt{mt}", name=f"vt{mt}")
                    nc.vector.tensor_copy(vt[:, :, D : D + 1], ones_sb[:, :, None])
                    pss = [
                        pmm.tile([P, 512], F32, tag="pb", name="ps")
                        for _ in range(2)
                    ]
                    for ct in range(CT):
                        for ci, (c0, csz) in enumerate(VCH):
                            nc.tensor.matmul(
                                pss[ci][:msz, :csz],
                                xT_b[ct][:, m0 : m0 + msz],
                                wv_sb[ct][:, c0 : c0 + csz],
                                start=(ct == 0),
                                stop=(ct == CT - 1),
                            )
                    for ci, (c0, csz) in enumerate(VCH):
                        nh = csz // D
                        h0 = c0 // D
                        nc.vector.tensor_tensor(
                            vt[:msz, h0 : h0 + nh, 0:D],
                            pss[ci][:msz, :csz].rearrange("p (h d) -> p h d", d=D),
                            bv_sb[:msz, c0 : c0 + csz].rearrange(
                                "p (h d) -> p h d", d=D
                            ),
                            ALU.add,
                        )
                    v_tok.append(vt)
                return v_tok

            def emit_qk(hp, xT_b):
                qt = qkp.tile([P, NP], BF16, tag="qk_q")
                ktb = qkp.tile([P, NP], BF16, tag="qk_k")
                for dst, ft in ((qt, hp), (ktb, CT + hp)):
                    pss = [
                        pmm.tile([P, 512], F32, tag="pb", name="ps")
                        for _ in range(2)
                    ]
                    for ct in range(CT):
                        for ci, (c0, csz) in enumerate(QCH):
                            nc.tensor.matmul(
                                pss[ci][:, :csz],
                                wqk_sb[ct][:, ft * P : (ft + 1) * P],
                                xT_b[ct][:, c0 : c0 + csz],
                                start=(ct == 0),
                                stop=(ct == CT - 1),
                            )
                    for ci, (c0, csz) in enumerate(QCH):
                        nc.vector.scalar_tensor_tensor(
                            dst[:, c0 : c0 + csz],
                            pss[ci][:, :csz],
                            1.0,
                            bqk_sb[:, ft : ft + 1].to_broadcast([P, csz]),
                            ALU.mult,
                            ALU.add,
                        )
                return qt, ktb

            def emit_scores(qt, ktb):
                """scores -> exp -> mask for both heads of the pair.
                Returns es0[hi] (kt0, dense) and esg[hi][grp] (packed
                windows) bf16 tiles."""
                es0 = {}
                esg = {0: [None, None], 1: [None, None]}
                mm = nc.tensor.matmul
                for hi in (0, 1):
                    es0[hi] = esp.tile([P, 512], BF16, tag="es0", name="es0")
                # kt0 vs q [0, 512): fills one psum bank per head
                for hi in (0, 1):
                    po = D * hi
                    sc = psc.tile([P, 512], F32, tag="sc", name="sc")
                    mm(
                        sc[:, :512],
                        ktb[po : po + D, 0:128],
                        qt[po : po + D, 0:512],
                        start=True,
                        stop=True,
                    )
                    nc.scalar.activation(es0[hi][:, :512], sc[:, :512], AF.Exp)
                    eng = nc.vector if hi == 0 else nc.gpsimd
                    eng.tensor_tensor(
                        es0[hi][:, 0:512], es0[hi][:, 0:512], mask0_sb[:, 0:512],
                        ALU.mult,
                    )
                # kt1..4 banded windows (2 per psum tile) + kt0 vs q [512, 578)
                # packed into the grp-0 tile
                for grp in (0, 1):
                    wins = WIN[2 * grp : 2 * grp + 2]
                    for hi in (0, 1):
                        po = D * hi
                        sc = psc.tile([P, 512], F32, tag="sc", name="sc")
                        for g, off, a, blen, k0, ksz in wins:
                            mm(
                                sc[:ksz, off + 2 : off + 2 + blen],
                                ktb[po : po + D, k0 : k0 + ksz],
                                qt[po : po + D, a : a + blen],
                                start=True, stop=True, skip_group_check=True,
                            )
                            mm(
                                sc[:ksz, off : off + 2],
                                ktb[po : po + D, k0 : k0 + ksz],
                                qt[po : po + D, 0:2],
                                start=True, stop=True, skip_group_check=True,
                            )
                        if grp == 0:
                            mm(
                                sc[:, K0B : K0B + 66],
                                ktb[po : po + D, 0:128],
                                qt[po : po + D, 512:NP],
                                start=True, stop=True, skip_group_check=True,
                            )
                        gw = GW[grp]
                        es = eswp.tile([P, GW[0]], BF16, tag=f"esg{grp}",
                                       name=f"esg{grp}")
                        esg[hi][grp] = es
                        nc.scalar.activation(es[:, :gw], sc[:, :gw], AF.Exp)
                        eng = nc.vector if hi == 0 else nc.gpsimd
                        m0 = grp * GW[0]
                        eng.tensor_tensor(
                            es[:, :gw], es[:, :gw],
                            maskw_sb[:, m0 : m0 + gw], ALU.mult,
                        )
                return es0, esg

            def emit_av(hp, hi, es0, esg, v_tok, oc_sb, srs):
                """banded attn@v for head h, stage output + denominator."""
                h = 2 * hp + hi
                oe = poe.tile([D + 1, NP + 2], F32, tag="oe", name="oe")
                mm = nc.tensor.matmul
                # kt0 dense: first writer of both PSUM banks (start=True)
                mm(oe[:, 0:512], v_tok[0][:, h, :], es0[hi][:, 0:512],
                   start=True, stop=False, skip_group_check=True)
                mm(oe[:, 512:NP], v_tok[0][:, h, :],
                   esg[hi][0][:, K0B : K0B + 66],
                   start=True, stop=False, skip_group_check=True)
                # banded tiles: accumulate segments (split at bank boundary)
                for wi, (grp, off, a, blen, k0, ksz) in enumerate(WIN):
                    es = esg[hi][grp]
                    vkt = v_tok[wi + 1][:ksz, h, :]
                    s0 = off + 2
                    if a + blen <= 512:
                        segs = [(s0, s0 + blen, a)]
                    else:
                        sp = s0 + (512 - a)
                        segs = [(s0, sp, a), (sp, s0 + blen, 512)]
                    for g0, g1, o0 in segs:
                        mm(oe[:, o0 : o0 + (g1 - g0)], vkt, es[:ksz, g0:g1],
                           start=False, stop=False, skip_group_check=True)
                    # CLS query column accumulates into output column 0
                    # (column 1 adds masked zeros)
                    mm(oe[:, 0:2], vkt, es[:ksz, off : off + 2],
                       start=False, stop=(wi == len(WIN) - 1),
                       skip_group_check=True)
                # head output rows -> oc (hi=1 shifts partitions via DMA);
                # extract fp32 denominator row
                if hi == 0:
                    nc.vector.tensor_copy(oc_sb[hp][0:D, 0:N], oe[0:D, 0:N])
                else:
                    stage = rcpp.tile([D, NP], BF16, tag="stage")
                    nc.vector.tensor_copy(stage[:, 0:N], oe[0:D, 0:N])
                    nc.sync.dma_start(oc_sb[hp][D : 2 * D, 0:N], stage[:, 0:N])
                srf = rcpp.tile([D + 1, NP], F32, tag="srf")
                nc.scalar.copy(srf[D : D + 1, 0:N], oe[D : D + 1, 0:N])
                nc.sync.dma_start(srs[hi : hi + 1, 0:N], srf[D : D + 1, 0:N])

            def emit_recip(srs):
                """reciprocal of the pair's denominators + broadcast DMA."""
                rr = rcpp.tile([2, NP], F32, tag="rr")
                nc.vector.reciprocal_approx_fast(rr[:, 0:N], srs[:, 0:N])
                rrd = drp.tile([2, NP], F32, tag="rrd")
                nc.sync.dma_start(rrd[:, :], rr[:, :])
                rb = rcpp.tile([P, NP], F32, tag="rb")
                nc.sync.dma_start(rb[0:D, 0:N], _bcast_ap(rrd[0][0:N], D))
                nc.sync.dma_start(rb[D : 2 * D, 0:N], _bcast_ap(rrd[1][0:N], D))
                return rb

            def emit_nmul(hp, rb, oc_sb):
                oc = oc_sb[hp]
                for po in (0, D):
                    nc.vector.tensor_tensor(
                        oc[po : po + D, 0:N],
                        oc[po : po + D, 0:N],
                        rb[po : po + D, 0:N],
                        ALU.mult,
                    )

            def emit_proj(b, oc_sb):
                for mt, (m0, msz) in enumerate(KT):
                    ysb = ysp.tile([P, C], F32, tag="ysb", name="ysb")
                    pss = [
                        pmm.tile([P, 512], F32, tag="pb", name="ps")
                        for _ in range(2)
                    ]
                    for ct in range(CT):
                        for ci, (c0, csz) in enumerate(VCH):
                            nc.tensor.matmul(
                                pss[ci][:msz, :csz],
                                oc_sb[ct][:, m0 : m0 + msz],
                                wp_sb[ct][:, c0 : c0 + csz],
                                start=(ct == 0),
                                stop=(ct == CT - 1),
                            )
                    for ci, (c0, csz) in enumerate(VCH):
                        nc.vector.tensor_tensor(
                            ysb[:msz, c0 : c0 + csz],
                            pss[ci][:msz, :csz],
                            bp_sb[:msz, c0 : c0 + csz],
                            ALU.add,
                        )
                    nc.sync.dma_start(
                        y[b * N + m0 : b * N + m0 + msz, :], ysb[:msz, :]
                    )

            # ---- main schedule (software-pipelined across pairs/batches) ----
            v_tok = emit_v(xT_b)
            # cover the wqk DMA wait and warm the clock for the first qk
            for i in range(28):
                ps = pmm.tile([P, 512], F32, tag="pb", name="ps")
                nc.tensor.matmul(ps[:, :512], wup[:, 0:P], wup[:, 0:512],
                                 start=True, stop=True, skip_group_check=True)
            for b in range(BPC):
                oc_sb = [
                    ocp.tile([P, NP], BF16, tag=f"oc{ct}", name=f"oc{ct}")
                    for ct in range(CT)
                ]
                srs_l = []
                rb_l = []
                qk_t = emit_qk(0, xT_b)
                nxt_x = None
                for hp in range(H // 2):
                    qt, ktb = qk_t
                    es0, esg = emit_scores(qt, ktb)
                    if hp == 2 and b + 1 < BPC:
                        nxt_x = emit_x_dma(b + 1)
                    if hp + 1 < H // 2:
                        qk_t = emit_qk(hp + 1, xT_b)
                    srs = rcpp.tile([2, NP], F32, tag="srs")
                    srs_l.append(srs)
                    if hp >= 1:
                        rb_l.append(emit_recip(srs_l[hp - 1]))
                    for hi in (0, 1):
                        emit_av(hp, hi, es0, esg, v_tok, oc_sb, srs)
                    if hp >= 1:
                        emit_nmul(hp - 1, rb_l[hp - 1], oc_sb)
                prev_oc = oc_sb
                rb = emit_recip(srs_l[H // 2 - 1])
                if b + 1 < BPC:
                    xT_b = nxt_x
                    v_tok = emit_v(xT_b)
                emit_nmul(H // 2 - 1, rb, prev_oc)
                emit_proj(b, prev_oc)

    nc.finalize()
    return nc


_CACHE = {}


def _make_in_maps(x, qkv_w, qkv_b, proj_w, proj_b):
    bf = ml_dtypes.bfloat16
    x = np.asarray(x, np.float32)
    qkv_w = np.asarray(qkv_w, np.float32)
    qkv_b = np.asarray(qkv_b, np.float32)
    proj_w = np.asarray(proj_w, np.float32)
    proj_b = np.asarray(proj_b, np.float32)

    wqkT = np.ascontiguousarray(qkv_w[: 2 * C].T).copy()
    wqkT[:, :C] *= SCALE
    wqkT = wqkT.astype(bf)
    wvT = np.ascontiguousarray(qkv_w[2 * C :].T).astype(bf)
    wpT = np.ascontiguousarray(proj_w.T).astype(bf)
    bqk_h = qkv_b[: 2 * C].copy()
    bqk_h[:C] *= SCALE
    bv_h = np.ascontiguousarray(qkv_b[2 * C :])

    m = np.zeros((NP, NP), np.float32)
    m[:N, :N] = _build_mask_np()
    mask0 = m[:P, :].astype(bf)
    maskw = np.zeros((P, MW), np.float32)
    for grp, off, a, blen, k0, ksz in WIN:
        base = grp * GW[0] + off
        maskw[:ksz, base] = 1.0
        maskw[:ksz, base + 1] = m[k0 : k0 + ksz, 1]
        maskw[:ksz, base + 2 : base + 2 + blen] = m[k0 : k0 + ksz, a : a + blen]
    maskw[:, K0B : K0B + 66] = m[:P, 512:NP]
    maskw = maskw.astype(bf)

    in_maps = []
    for c in range(NCORES):
        xp_c = np.zeros((BPC, NP, C), np.float32)
        xp_c[:, :N, :] = x[c * BPC : (c + 1) * BPC]
        xT_c = np.ascontiguousarray(xp_c.reshape(TP, C).T).astype(bf)
        in_maps.append(
            {
                "xT": xT_c,
                "wqkT": wqkT,
                "wvT": wvT,
                "wpT": wpT,
                "bqk": bqk_h,
                "bv": bv_h,
                "bp": proj_b,
                "mask0d": mask0,
                "maskwd": maskw,
                "ones12": np.ones(H, bf),
            }
        )
    return in_maps


def kernel(x, qkv_w, qkv_b, proj_w, proj_b):
    if "nc" not in _CACHE:
        _CACHE["nc"] = _build_program()
    nc = _CACHE["nc"]

    in_maps = _make_in_maps(x, qkv_w, qkv_b, proj_w, proj_b)
    res = run_bass_kernel_spmd(nc, in_maps, list(range(NCORES)))
    out = np.concatenate(
        [res.results[c]["y"].reshape(BPC, N, C) for c in range(NCORES)], axis=0
    )
    return out.astype(np.float32)


# revision 27
# speedup vs baseline: 1.1058x; 1.1058x over previous
"""Trainium2 Bass kernel for CustomAttention (ViT-style windowed attention).

Math (per batch element):
  qkv = x @ qkv_w.T + qkv_b            -> q, k, v  [H=12 heads, D=64]
  s   = (q * D^-0.5) @ k.T             masked by a fixed 24x24-grid window
  attn = softmax(s)                    (CLS row/col always attended)
  out  = attn @ v                      -> concat heads -> @ proj_w.T + proj_b

Sharding: data-parallel over batch across 8 cores (4 images/core).

Key device-side choices:
  - The window mask in row-major token order is a band: patch key j is
    attended only by queries in [j-25, j+25] (plus CLS row/col). Scores and
    attn@v therefore run BANDED per 128-key tile: each key tile streams only
    its ~180-column query window instead of all 578. Key tile 0 keeps the
    full query range (its row 0 is the CLS key, attended by every query).
    The CLS query (attends all keys) lands in column 0 of each window via
    N=1 matmuls; its attn@v contributions accumulate into output column 0.
  - Window score tiles are packed pairwise into one PSUM bank (kt1+kt2,
    kt3+kt4) so exp and mask run once per packed tile.
  - attn@v accumulates banded segments into one [65, 580] PSUM tile spanning
    2 banks; matmuls split at the 512-column bank boundary and the first
    writer of each bank uses start=True (clears has_written for the bank).
  - All matmul operands are bf16 (fp32 PSUM accumulation): bf16 streams
    1 col/cycle at any N (fp32r needs N>=256), enables fast weight load, and
    halves DMA/SBUF. Measured max-rel error ~4e-3 vs the 2e-2 gate.
  - softmax runs unnormalized (no max subtraction; |s| small): exp then
    mask-multiply. v carries an interleaved ones column so attn@v also
    yields the softmax denominators (row 64). Head outputs + denominators
    stage through one bf16 copy; normalization is deferred one pair:
    Scalar-engine reciprocal, DRAM-source partition-broadcast DMA, bf16
    multiply before proj.
  - a burst of junk warmup matmuls at kernel start keeps the PE HAM clock
    gate at full rate while x/weights stream in (x DMAs issued first).
"""

import ml_dtypes
import numpy as np

import concourse.bass as bass
import concourse.mybir as mybir
from concourse import bacc
from concourse.bass_utils import run_bass_kernel_spmd
from concourse.tile import TileContext

B, N, C = 32, 577, 768
H, D = 12, 64
NCORES = 8
BPC = B // NCORES            # batches per core
NP = N + 1                   # padded token count (even)
TP = BPC * NP
T = BPC * N
SCALE = float(D) ** -0.5     # 0.125, exact in bf16
F32 = mybir.dt.float32
BF16 = mybir.dt.bfloat16
P = 128

CT = C // P                                      # 6 contraction tiles
KT = [(0, 128), (128, 128), (256, 128), (384, 128), (512, 65)]
QCH = [(0, 290), (290, 288)]                     # qk / kt0-score chunks
VCH = [(0, 512), (512, 256)]                     # v / proj output chunks
# banded windows for key tiles 1..4: (grp, off, a, blen, k0, ksz)
# grp selects the packed psum/es tile (0: kt1+kt2, 1: kt3+kt4); within it,
# cols off+0/off+1 = scores vs q tokens 0 (CLS) and 1 (masked to zero),
# cols off+2..off+1+blen = q tokens [a, a+blen). All col offsets/sizes even.
WIN = [
    (0, 0, 102, 180, 128, 128),
    (0, 182, 230, 180, 256, 128),
    (1, 0, 358, 180, 384, 128),
    (1, 182, 486, 92, 512, 65),
]
# key tile 0's scores vs q tokens [512, 578) also pack into the grp-0 tile
# at cols [K0B, K0B+66) (its [0, 512) chunk fills a whole bank on its own).
K0B = 364
GW = [430, 276]              # packed window tile widths
MW = sum(GW)                 # banded mask tile width
AF = mybir.ActivationFunctionType
ALU = mybir.AluOpType


def _build_mask_np():
    img = 24
    p = np.arange(img * img)
    pi, pj = p // img, p % img
    ok = (np.abs(pi[:, None] - pi[None, :]) <= 1) & (
        np.abs(pj[:, None] - pj[None, :]) <= 1
    )
    m = np.zeros((N, N), dtype=np.float32)
    m[1:, 1:] = ok
    m[0, :] = True
    m[:, 0] = True
    return m


def _bcast_ap(ap1d, parts):
    """1-row AP -> [parts, n] with partition stride 0 (DRAM-source DMA)."""
    return bass.AP(
        tensor=ap1d.tensor, offset=ap1d.offset, ap=[[0, parts]] + list(ap1d.ap)[-1:]
    )


def _build_program():
    nc = bacc.Bacc("TRN2", target_bir_lowering=False, debug=False)
    xT = nc.dram_tensor("xT", [C, TP], BF16, kind="ExternalInput").ap()
    wqkT = nc.dram_tensor("wqkT", [C, 2 * C], BF16, kind="ExternalInput").ap()
    wvT = nc.dram_tensor("wvT", [C, C], BF16, kind="ExternalInput").ap()
    wpT = nc.dram_tensor("wpT", [C, C], BF16, kind="ExternalInput").ap()
    bqk = nc.dram_tensor("bqk", [2 * C], F32, kind="ExternalInput").ap()
    bv = nc.dram_tensor("bv", [C], F32, kind="ExternalInput").ap()
    bp = nc.dram_tensor("bp", [C], F32, kind="ExternalInput").ap()
    mask0d = nc.dram_tensor("mask0d", [P, NP], BF16, kind="ExternalInput").ap()
    maskwd = nc.dram_tensor("maskwd", [P, MW], BF16, kind="ExternalInput").ap()
    ones12 = nc.dram_tensor("ones12", [H], BF16, kind="ExternalInput").ap()
    y = nc.dram_tensor("y", [T, C], F32, kind="ExternalOutput").ap()

    with TileContext(nc) as tc:
        with (
            tc.tile_pool(name="singles", bufs=1) as singles,
            tc.tile_pool(name="xp", bufs=2) as xp,
            tc.tile_pool(name="qkp", bufs=2) as qkp,
            tc.tile_pool(name="vtp", bufs=2) as vtp,
            tc.tile_pool(name="ocp", bufs=2) as ocp,
            tc.tile_pool(name="esp", bufs=2) as esp,
            tc.tile_pool(name="esw", bufs=4) as eswp,
            tc.tile_pool(name="rcp", bufs=3) as rcpp,
            tc.tile_pool(name="ysp", bufs=2) as ysp,
            tc.tile_pool(name="pmm", bufs=3, space="PSUM") as pmm,
            tc.tile_pool(name="psc", bufs=3, space="PSUM") as psc,
            tc.tile_pool(name="poe", bufs=1, space="PSUM") as poe,
            tc.tile_pool(name="drp", bufs=4, space="DRAM") as drp,
        ):
            # ---- prefetch batch 0's x before the weights ----
            def emit_x_dma(b):
                xT_b = []
                for ct in range(CT):
                    t = xp.tile([P, NP], BF16, tag=f"x{ct}", name=f"x{ct}")
                    nc.sync.dma_start(
                        t[:], xT[ct * P : (ct + 1) * P, b * NP : (b + 1) * NP]
                    )
                    xT_b.append(t)
                return xT_b

            xT_b = emit_x_dma(0)

            # ---- PE warmup: junk matmuls overlap the input DMAs and get
            # the HAM clock gate to 8/8 before real work starts ----
            wup = singles.tile([P, 512], BF16, tag="wup")
            nc.vector.memset(wup[:], 1.0)
            for i in range(24):
                ps = pmm.tile([P, 512], F32, tag="pb", name="ps")
                nc.tensor.matmul(ps[:, :512], wup[:, 0:P], wup[:, 0:512],
                                 start=True, stop=True, skip_group_check=True)

            # ---- persistent loads (v/qk weights first) ----
            wv_sb = []
            wqk_sb = []
            wp_sb = []
            for ct in range(CT):
                t = singles.tile([P, C], BF16, tag=f"wv{ct}")
                nc.sync.dma_start(t[:], wvT[ct * P : (ct + 1) * P, :])
                wv_sb.append(t)
            for ct in range(CT):
                t = singles.tile([P, 2 * C], BF16, tag=f"wqk{ct}")
                nc.sync.dma_start(t[:], wqkT[ct * P : (ct + 1) * P, :])
                wqk_sb.append(t)
            bqk_sb = singles.tile([P, 2 * C // P], F32, tag="bqk")
            nc.sync.dma_start(bqk_sb[:], bqk.rearrange("(o p) -> p o", p=P))
            bv_sb = singles.tile([P, C], F32, tag="bv")
            nc.sync.dma_start(bv_sb[:], _bcast_ap(bv, P))
            ones_sb = singles.tile([P, H], BF16, tag="ones_sb")
            nc.sync.dma_start(ones_sb[:], _bcast_ap(ones12, P))
            mask0_sb = singles.tile([P, NP], BF16, tag="mask0")
            nc.sync.dma_start(mask0_sb[:], mask0d[:, :])
            maskw_sb = singles.tile([P, MW], BF16, tag="maskw")
            nc.sync.dma_start(maskw_sb[:], maskwd[:, :])
            for ct in range(CT):
                t = singles.tile([P, C], BF16, tag=f"wp{ct}")
                nc.sync.dma_start(t[:], wpT[ct * P : (ct + 1) * P, :])
                wp_sb.append(t)
            bp_sb = singles.tile([P, C], F32, tag="bp")
            nc.sync.dma_start(bp_sb[:], _bcast_ap(bp, P))

            def emit_v(xT_b):
                v_tok = []
                for mt, (m0, msz) in enumerate(KT):
                    vt = vtp.tile([P, H, D + 1], BF16, tag=f"vt{mt}", name=f"vt{mt}")
                    nc.vector.tensor_copy(vt[:, :, D : D + 1], ones_sb[:, :, None])
                    pss = [
                        pmm.tile([P, 512], F32, tag="pb", name="ps")
                        for _ in range(2)
                    ]
                    for ct in range(CT):
                        for ci, (c0, csz) in enumerate(VCH):
                            nc.tensor.matmul(
                                pss[ci][:msz, :csz],
                                xT_b[ct][:, m0 : m0 + msz],
                                wv_sb[ct][:, c0 : c0 + csz],
                                start=(ct == 0),
                                stop=(ct == CT - 1),
                            )
                    for ci, (c0, csz) in enumerate(VCH):
                        nh = csz // D
                        h0 = c0 // D
                        nc.vector.tensor_tensor(
                            vt[:msz, h0 : h0 + nh, 0:D],
                            pss[ci][:msz, :csz].rearrange("p (h d) -> p h d", d=D),
                            bv_sb[:msz, c0 : c0 + csz].rearrange(
                                "p (h d) -> p h d", d=D
                            ),
                            ALU.add,
                        )
                    v_tok.append(vt)
                return v_tok

            def emit_qk(hp, xT_b):
                qt = qkp.tile([P, NP], BF16, tag="qk_q")
                ktb = qkp.tile([P, NP], BF16, tag="qk_k")
                for dst, ft in ((qt, hp), (ktb, CT + hp)):
                    pss = [
                        pmm.tile([P, 512], F32, tag="pb", name="ps")
                        for _ in range(2)
                    ]
                    for ct in range(CT):
                        for ci, (c0, csz) in enumerate(QCH):
                            nc.tensor.matmul(
                                pss[ci][:, :csz],
                                wqk_sb[ct][:, ft * P : (ft + 1) * P],
                                xT_b[ct][:, c0 : c0 + csz],
                                start=(ct == 0),
                                stop=(ct == CT - 1),
                            )
                    for ci, (c0, csz) in enumerate(QCH):
                        nc.vector.scalar_tensor_tensor(
                            dst[:, c0 : c0 + csz],
                            pss[ci][:, :csz],
                            1.0,
                            bqk_sb[:, ft : ft + 1].to_broadcast([P, csz]),
                            ALU.mult,
                            ALU.add,
                        )
                return qt, ktb

            def emit_scores(qt, ktb):
                """scores -> exp -> mask for both heads of the pair.
                Returns es0[hi] (kt0, dense) and esg[hi][grp] (packed
                windows) bf16 tiles."""
                es0 = {}
                esg = {0: [None, None], 1: [None, None]}
                mm = nc.tensor.matmul
                for hi in (0, 1):
                    es0[hi] = esp.tile([P, 512], BF16, tag="es0", name="es0")
                # kt0 vs q [0, 512): fills one psum bank per head
                for hi in (0, 1):
                    po = D * hi
                    sc = psc.tile([P, 512], F32, tag="sc", name="sc")
                    mm(
                        sc[:, :512],
                        ktb[po : po + D, 0:128],
                        qt[po : po + D, 0:512],
                        start=True,
                        stop=True,
                    )
                    nc.scalar.activation(es0[hi][:, :512], sc[:, :512], AF.Exp)
                    eng = nc.vector if hi == 0 else nc.gpsimd
                    eng.tensor_tensor(
                        es0[hi][:, 0:512], es0[hi][:, 0:512], mask0_sb[:, 0:512],
                        ALU.mult,
                    )
                # kt1..4 banded windows (2 per psum tile) + kt0 vs q [512, 578)
                # packed into the grp-0 tile
                for grp in (0, 1):
                    wins = WIN[2 * grp : 2 * grp + 2]
                    for hi in (0, 1):
                        po = D * hi
                        sc = psc.tile([P, 512], F32, tag="sc", name="sc")
                        for g, off, a, blen, k0, ksz in wins:
                            mm(
                                sc[:ksz, off + 2 : off + 2 + blen],
                                ktb[po : po + D, k0 : k0 + ksz],
                                qt[po : po + D, a : a + blen],
                                start=True, stop=True, skip_group_check=True,
                            )
                            mm(
                                sc[:ksz, off : off + 2],
                                ktb[po : po + D, k0 : k0 + ksz],
                                qt[po : po + D, 0:2],
                                start=True, stop=True, skip_group_check=True,
                            )
                        if grp == 0:
                            mm(
                                sc[:, K0B : K0B + 66],
                                ktb[po : po + D, 0:128],
                                qt[po : po + D, 512:NP],
                                start=True, stop=True, skip_group_check=True,
                            )
                        gw = GW[grp]
                        es = eswp.tile([P, GW[0]], BF16, tag=f"esg{grp}",
                                       name=f"esg{grp}")
                        esg[hi][grp] = es
                        nc.scalar.activation(es[:, :gw], sc[:, :gw], AF.Exp)
                        eng = nc.vector if hi == 0 else nc.gpsimd
                        m0 = grp * GW[0]
                        eng.tensor_tensor(
                            es[:, :gw], es[:, :gw],
                            maskw_sb[:, m0 : m0 + gw], ALU.mult,
                        )
                return es0, esg

            def emit_av(hp, hi, es0, esg, v_tok, oc_sb, srs):
                """banded attn@v for head h, stage output + denominator."""
                h = 2 * hp + hi
                oe = poe.tile([D + 1, NP + 2], F32, tag="oe", name="oe")
                mm = nc.tensor.matmul
                # kt0 dense: first writer of both PSUM banks (start=True)
                mm(oe[:, 0:512], v_tok[0][:, h, :], es0[hi][:, 0:512],
                   start=True, stop=False, skip_group_check=True)
                mm(oe[:, 512:NP], v_tok[0][:, h, :],
                   esg[hi][0][:, K0B : K0B + 66],
                   start=True, stop=False, skip_group_check=True)
                # banded tiles: accumulate segments (split at bank boundary)
                for wi, (grp, off, a, blen, k0, ksz) in enumerate(WIN):
                    es = esg[hi][grp]
                    vkt = v_tok[wi + 1][:ksz, h, :]
                    s0 = off + 2
                    if a + blen <= 512:
                        segs = [(s0, s0 + blen, a)]
                    else:
                        sp = s0 + (512 - a)
                        segs = [(s0, sp, a), (sp, s0 + blen, 512)]
                    for g0, g1, o0 in segs:
                        mm(oe[:, o0 : o0 + (g1 - g0)], vkt, es[:ksz, g0:g1],
                           start=False, stop=False, skip_group_check=True)
                    # CLS query column accumulates into output column 0
                    # (column 1 adds masked zeros)
                    mm(oe[:, 0:2], vkt, es[:ksz, off : off + 2],
                       start=False, stop=(wi == len(WIN) - 1),
                       skip_group_check=True)
                # head output rows -> oc (hi=1 shifts partitions via DMA);
                # extract fp32 denominator row
                if hi == 0:
                    nc.vector.tensor_copy(oc_sb[hp][0:D, 0:N], oe[0:D, 0:N])
                else:
                    stage = rcpp.tile([D, NP], BF16, tag="stage")
                    nc.vector.tensor_copy(stage[:, 0:N], oe[0:D, 0:N])
                    nc.sync.dma_start(oc_sb[hp][D : 2 * D, 0:N], stage[:, 0:N])
                srf = rcpp.tile([D + 1, NP], F32, tag="srf")
                nc.scalar.copy(srf[D : D + 1, 0:N], oe[D : D + 1, 0:N])
                nc.sync.dma_start(srs[hi : hi + 1, 0:N], srf[D : D + 1, 0:N])

            def emit_recip(srs):
                """reciprocal of the pair's denominators + broadcast DMA."""
                rr = rcpp.tile([2, NP], F32, tag="rr")
                nc.vector.reciprocal_approx_fast(rr[:, 0:N], srs[:, 0:N])
                rrd = drp.tile([2, NP], F32, tag="rrd")
                nc.sync.dma_start(rrd[:, :], rr[:, :])
                rb = rcpp.tile([P, NP], F32, tag="rb")
                nc.sync.dma_start(rb[0:D, 0:N], _bcast_ap(rrd[0][0:N], D))
                nc.sync.dma_start(rb[D : 2 * D, 0:N], _bcast_ap(rrd[1][0:N], D))
                return rb

            def emit_nmul(hp, rb, oc_sb):
                oc = oc_sb[hp]
                for po in (0, D):
                    nc.vector.tensor_tensor(
                        oc[po : po + D, 0:N],
                        oc[po : po + D, 0:N],
                        rb[po : po + D, 0:N],
                        ALU.mult,
                    )

            def emit_proj(b, oc_sb):
                for mt, (m0, msz) in enumerate(KT):
                    ysb = ysp.tile([P, C], F32, tag="ysb", name="ysb")
                    pss = [
                        pmm.tile([P, 512], F32, tag="pb", name="ps")
                        for _ in range(2)
                    ]
                    for ct in range(CT):
                        for ci, (c0, csz) in enumerate(VCH):
                            nc.tensor.matmul(
                                pss[ci][:msz, :csz],
                                oc_sb[ct][:, m0 : m0 + msz],
                                wp_sb[ct][:, c0 : c0 + csz],
                                start=(ct == 0),
                                stop=(ct == CT - 1),
                            )
                    for ci, (c0, csz) in enumerate(VCH):
                        nc.vector.tensor_tensor(
                            ysb[:msz, c0 : c0 + csz],
                            pss[ci][:msz, :csz],
                            bp_sb[:msz, c0 : c0 + csz],
                            ALU.add,
                        )
                    nc.sync.dma_start(
                        y[b * N + m0 : b * N + m0 + msz, :], ysb[:msz, :]
                    )

            # ---- main schedule (software-pipelined across pairs/batches) ----
            v_tok = emit_v(xT_b)
            # cover the wqk DMA wait and warm the clock for the first qk
            for i in range(28):
                ps = pmm.tile([P, 512], F32, tag="pb", name="ps")
                nc.tensor.matmul(ps[:, :512], wup[:, 0:P], wup[:, 0:512],
                                 start=True, stop=True, skip_group_check=True)
            for b in range(BPC):
                oc_sb = [
                    ocp.tile([P, NP], BF16, tag=f"oc{ct}", name=f"oc{ct}")
                    for ct in range(CT)
                ]
                srs_l = []
                rb_l = []
                qk_t = emit_qk(0, xT_b)
                nxt_x = None
                for hp in range(H // 2):
                    qt, ktb = qk_t
                    es0, esg = emit_scores(qt, ktb)
                    if hp == 2 and b + 1 < BPC:
                        nxt_x = emit_x_dma(b + 1)
                    if hp + 1 < H // 2:
                        qk_t = emit_qk(hp + 1, xT_b)
                    srs = rcpp.tile([2, NP], F32, tag="srs")
                    srs_l.append(srs)
                    if hp >= 1:
                        rb_l.append(emit_recip(srs_l[hp - 1]))
                    for hi in (0, 1):
                        emit_av(hp, hi, es0, esg, v_tok, oc_sb, srs)
                    if hp >= 1:
                        emit_nmul(hp - 1, rb_l[hp - 1], oc_sb)
                prev_oc = oc_sb
                rb = emit_recip(srs_l[H // 2 - 1])
                if b + 1 < BPC:
                    xT_b = nxt_x
                    v_tok = emit_v(xT_b)
                emit_nmul(H // 2 - 1, rb, prev_oc)
                emit_proj(b, prev_oc)

    nc.finalize()
    return nc


_CACHE = {}


def _make_in_maps(x, qkv_w, qkv_b, proj_w, proj_b):
    bf = ml_dtypes.bfloat16
    x = np.asarray(x, np.float32)
    qkv_w = np.asarray(qkv_w, np.float32)
    qkv_b = np.asarray(qkv_b, np.float32)
    proj_w = np.asarray(proj_w, np.float32)
    proj_b = np.asarray(proj_b, np.float32)

    wqkT = np.ascontiguousarray(qkv_w[: 2 * C].T).copy()
    wqkT[:, :C] *= SCALE
    wqkT = wqkT.astype(bf)
    wvT = np.ascontiguousarray(qkv_w[2 * C :].T).astype(bf)
    wpT = np.ascontiguousarray(proj_w.T).astype(bf)
    bqk_h = qkv_b[: 2 * C].copy()
    bqk_h[:C] *= SCALE
    bv_h = np.ascontiguousarray(qkv_b[2 * C :])

    m = np.zeros((NP, NP), np.float32)
    m[:N, :N] = _build_mask_np()
    mask0 = m[:P, :].astype(bf)
    maskw = np.zeros((P, MW), np.float32)
    for grp, off, a, blen, k0, ksz in WIN:
        base = grp * GW[0] + off
        maskw[:ksz, base] = 1.0
        maskw[:ksz, base + 1] = m[k0 : k0 + ksz, 1]
        maskw[:ksz, base + 2 : base + 2 + blen] = m[k0 : k0 + ksz, a : a + blen]
    maskw[:, K0B : K0B + 66] = m[:P, 512:NP]
    maskw = maskw.astype(bf)

    in_maps = []
    for c in range(NCORES):
        xp_c = np.zeros((BPC, NP, C), np.float32)
        xp_c[:, :N, :] = x[c * BPC : (c + 1) * BPC]
        xT_c = np.ascontiguousarray(xp_c.reshape(TP, C).T).astype(bf)
        in_maps.append(
            {
                "xT": xT_c,
                "wqkT": wqkT,
                "wvT": wvT,
                "wpT": wpT,
                "bqk": bqk_h,
                "bv": bv_h,
                "bp": proj_b,
                "mask0d": mask0,
                "maskwd": maskw,
                "ones12": np.ones(H, bf),
            }
        )
    return in_maps


def kernel(x, qkv_w, qkv_b, proj_w, proj_b):
    if "nc" not in _CACHE:
        _CACHE["nc"] = _build_program()
    nc = _CACHE["nc"]

    in_maps = _make_in_maps(x, qkv_w, qkv_b, proj_w, proj_b)
    res = run_bass_kernel_spmd(nc, in_maps, list(range(NCORES)))
    out = np.concatenate(
        [res.results[c]["y"].reshape(BPC, N, C) for c in range(NCORES)], axis=0
    )
    return out.astype(np.float32)


# revision 28
# speedup vs baseline: 1.1090x; 1.0029x over previous
"""Trainium2 Bass kernel for CustomAttention (ViT-style windowed attention).

Math (per batch element):
  qkv = x @ qkv_w.T + qkv_b            -> q, k, v  [H=12 heads, D=64]
  s   = (q * D^-0.5) @ k.T             masked by a fixed 24x24-grid window
  attn = softmax(s)                    (CLS row/col always attended)
  out  = attn @ v                      -> concat heads -> @ proj_w.T + proj_b

Sharding: data-parallel over batch across 8 cores (4 images/core).

Key device-side choices:
  - The window mask in row-major token order is a band: patch key j is
    attended only by queries in [j-25, j+25] (plus CLS row/col). Scores and
    attn@v therefore run BANDED per 128-key tile: each key tile streams only
    its ~180-column query window instead of all 578. Key tile 0 keeps the
    full query range (its row 0 is the CLS key, attended by every query).
    The CLS query (attends all keys) lands in column 0 of each window via
    N=1 matmuls; its attn@v contributions accumulate into output column 0.
  - Window score tiles are packed pairwise into one PSUM bank (kt1+kt2,
    kt3+kt4) so exp and mask run once per packed tile.
  - attn@v accumulates banded segments into one [65, 580] PSUM tile spanning
    2 banks; matmuls split at the 512-column bank boundary and the first
    writer of each bank uses start=True (clears has_written for the bank).
  - All matmul operands are bf16 (fp32 PSUM accumulation): bf16 streams
    1 col/cycle at any N (fp32r needs N>=256), enables fast weight load, and
    halves DMA/SBUF. Measured max-rel error ~4e-3 vs the 2e-2 gate.
  - softmax runs unnormalized (no max subtraction; |s| small): exp then
    mask-multiply. v carries an interleaved ones column so attn@v also
    yields the softmax denominators (row 64). Head outputs + denominators
    stage through one bf16 copy; normalization is deferred one pair:
    Scalar-engine reciprocal, DRAM-source partition-broadcast DMA, bf16
    multiply before proj.
  - a burst of junk warmup matmuls at kernel start keeps the PE HAM clock
    gate at full rate while x/weights stream in (x DMAs issued first).
"""

import ml_dtypes
import numpy as np

import concourse.bass as bass
import concourse.mybir as mybir
from concourse import bacc
from concourse.bass_utils import run_bass_kernel_spmd
from concourse.tile import TileContext

B, N, C = 32, 577, 768
H, D = 12, 64
NCORES = 8
BPC = B // NCORES            # batches per core
NP = N + 1                   # padded token count (even)
TP = BPC * NP
T = BPC * N
SCALE = float(D) ** -0.5     # 0.125, exact in bf16
F32 = mybir.dt.float32
BF16 = mybir.dt.bfloat16
P = 128

CT = C // P                                      # 6 contraction tiles
KT = [(0, 128), (128, 128), (256, 128), (384, 128), (512, 65)]
QCH = [(0, 290), (290, 288)]                     # qk / kt0-score chunks
VCH = [(0, 512), (512, 256)]                     # v / proj output chunks
# banded windows for key tiles 1..4: (grp, off, a, blen, k0, ksz)
# grp selects the packed psum/es tile (0: kt1+kt2, 1: kt3+kt4); within it,
# cols off+0/off+1 = scores vs q tokens 0 (CLS) and 1 (masked to zero),
# cols off+2..off+1+blen = q tokens [a, a+blen). All col offsets/sizes even.
WIN = [
    (0, 0, 102, 180, 128, 128),
    (0, 182, 230, 180, 256, 128),
    (1, 0, 358, 180, 384, 128),
    (1, 182, 486, 92, 512, 65),
]
# key tile 0's scores vs q tokens [512, 578) also pack into the grp-0 tile
# at cols [K0B, K0B+66) (its [0, 512) chunk fills a whole bank on its own).
K0B = 364
GW = [430, 276]              # packed window tile widths
MW = sum(GW)                 # banded mask tile width
AF = mybir.ActivationFunctionType
ALU = mybir.AluOpType


def _build_mask_np():
    img = 24
    p = np.arange(img * img)
    pi, pj = p // img, p % img
    ok = (np.abs(pi[:, None] - pi[None, :]) <= 1) & (
        np.abs(pj[:, None] - pj[None, :]) <= 1
    )
    m = np.zeros((N, N), dtype=np.float32)
    m[1:, 1:] = ok
    m[0, :] = True
    m[:, 0] = True
    return m


def _bcast_ap(ap1d, parts):
    """1-row AP -> [parts, n] with partition stride 0 (DRAM-source DMA)."""
    return bass.AP(
        tensor=ap1d.tensor, offset=ap1d.offset, ap=[[0, parts]] + list(ap1d.ap)[-1:]
    )


def _build_program():
    nc = bacc.Bacc("TRN2", target_bir_lowering=False, debug=False)
    xT = nc.dram_tensor("xT", [C, TP], BF16, kind="ExternalInput").ap()
    wqkT = nc.dram_tensor("wqkT", [C, 2 * C], BF16, kind="ExternalInput").ap()
    wvT = nc.dram_tensor("wvT", [C, C], BF16, kind="ExternalInput").ap()
    wpT = nc.dram_tensor("wpT", [C, C], BF16, kind="ExternalInput").ap()
    bqk = nc.dram_tensor("bqk", [2 * C], F32, kind="ExternalInput").ap()
    bv = nc.dram_tensor("bv", [C], F32, kind="ExternalInput").ap()
    bp = nc.dram_tensor("bp", [C], F32, kind="ExternalInput").ap()
    mask0d = nc.dram_tensor("mask0d", [P, NP], BF16, kind="ExternalInput").ap()
    maskwd = nc.dram_tensor("maskwd", [P, MW], BF16, kind="ExternalInput").ap()
    ones12 = nc.dram_tensor("ones12", [H], BF16, kind="ExternalInput").ap()
    y = nc.dram_tensor("y", [T, C], F32, kind="ExternalOutput").ap()

    with TileContext(nc) as tc:
        with (
            tc.tile_pool(name="singles", bufs=1) as singles,
            tc.tile_pool(name="xp", bufs=2) as xp,
            tc.tile_pool(name="qkp", bufs=2) as qkp,
            tc.tile_pool(name="vtp", bufs=2) as vtp,
            tc.tile_pool(name="ocp", bufs=2) as ocp,
            tc.tile_pool(name="esp", bufs=2) as esp,
            tc.tile_pool(name="esw", bufs=4) as eswp,
            tc.tile_pool(name="rcp", bufs=3) as rcpp,
            tc.tile_pool(name="ysp", bufs=2) as ysp,
            tc.tile_pool(name="pmm", bufs=2, space="PSUM") as pmm,
            tc.tile_pool(name="psc", bufs=4, space="PSUM") as psc,
            tc.tile_pool(name="poe", bufs=1, space="PSUM") as poe,
            tc.tile_pool(name="drp", bufs=4, space="DRAM") as drp,
        ):
            # ---- prefetch batch 0's x before the weights ----
            def emit_x_dma(b):
                xT_b = []
                for ct in range(CT):
                    t = xp.tile([P, NP], BF16, tag=f"x{ct}", name=f"x{ct}")
                    nc.sync.dma_start(
                        t[:], xT[ct * P : (ct + 1) * P, b * NP : (b + 1) * NP]
                    )
                    xT_b.append(t)
                return xT_b

            xT_b = emit_x_dma(0)

            # ---- PE warmup: junk matmuls overlap the input DMAs and get
            # the HAM clock gate to 8/8 before real work starts ----
            wup = singles.tile([P, 512], BF16, tag="wup")
            nc.vector.memset(wup[:], 1.0)
            for i in range(24):
                ps = pmm.tile([P, 512], F32, tag="pb", name="ps")
                nc.tensor.matmul(ps[:, :512], wup[:, 0:P], wup[:, 0:512],
                                 start=True, stop=True, skip_group_check=True)

            # ---- persistent loads (v/qk weights first) ----
            wv_sb = []
            wqk_sb = []
            wp_sb = []
            for ct in range(CT):
                t = singles.tile([P, C], BF16, tag=f"wv{ct}")
                nc.sync.dma_start(t[:], wvT[ct * P : (ct + 1) * P, :])
                wv_sb.append(t)
            for ct in range(CT):
                t = singles.tile([P, 2 * C], BF16, tag=f"wqk{ct}")
                nc.sync.dma_start(t[:], wqkT[ct * P : (ct + 1) * P, :])
                wqk_sb.append(t)
            bqk_sb = singles.tile([P, 2 * C // P], F32, tag="bqk")
            nc.sync.dma_start(bqk_sb[:], bqk.rearrange("(o p) -> p o", p=P))
            bv_sb = singles.tile([P, C], F32, tag="bv")
            nc.sync.dma_start(bv_sb[:], _bcast_ap(bv, P))
            ones_sb = singles.tile([P, H], BF16, tag="ones_sb")
            nc.sync.dma_start(ones_sb[:], _bcast_ap(ones12, P))
            mask0_sb = singles.tile([P, NP], BF16, tag="mask0")
            nc.sync.dma_start(mask0_sb[:], mask0d[:, :])
            maskw_sb = singles.tile([P, MW], BF16, tag="maskw")
            nc.sync.dma_start(maskw_sb[:], maskwd[:, :])
            for ct in range(CT):
                t = singles.tile([P, C], BF16, tag=f"wp{ct}")
                nc.sync.dma_start(t[:], wpT[ct * P : (ct + 1) * P, :])
                wp_sb.append(t)
            bp_sb = singles.tile([P, C], F32, tag="bp")
            nc.sync.dma_start(bp_sb[:], _bcast_ap(bp, P))

            def emit_v(xT_b):
                v_tok = []
                for mt, (m0, msz) in enumerate(KT):
                    vt = vtp.tile([P, H, D + 1], BF16, tag=f"vt{mt}", name=f"vt{mt}")
                    nc.vector.tensor_copy(vt[:, :, D : D + 1], ones_sb[:, :, None])
                    pss = [
                        pmm.tile([P, 512], F32, tag="pb", name="ps")
                        for _ in range(2)
                    ]
                    for ct in range(CT):
                        for ci, (c0, csz) in enumerate(VCH):
                            nc.tensor.matmul(
                                pss[ci][:msz, :csz],
                                xT_b[ct][:, m0 : m0 + msz],
                                wv_sb[ct][:, c0 : c0 + csz],
                                start=(ct == 0),
                                stop=(ct == CT - 1),
                            )
                    for ci, (c0, csz) in enumerate(VCH):
                        nh = csz // D
                        h0 = c0 // D
                        nc.vector.tensor_tensor(
                            vt[:msz, h0 : h0 + nh, 0:D],
                            pss[ci][:msz, :csz].rearrange("p (h d) -> p h d", d=D),
                            bv_sb[:msz, c0 : c0 + csz].rearrange(
                                "p (h d) -> p h d", d=D
                            ),
                            ALU.add,
                        )
                    v_tok.append(vt)
                return v_tok

            def emit_qk(hp, xT_b):
                qt = qkp.tile([P, NP], BF16, tag="qk_q")
                ktb = qkp.tile([P, NP], BF16, tag="qk_k")
                for dst, ft in ((qt, hp), (ktb, CT + hp)):
                    pss = [
                        pmm.tile([P, 512], F32, tag="pb", name="ps")
                        for _ in range(2)
                    ]
                    for ct in range(CT):
                        for ci, (c0, csz) in enumerate(QCH):
                            nc.tensor.matmul(
                                pss[ci][:, :csz],
                                wqk_sb[ct][:, ft * P : (ft + 1) * P],
                                xT_b[ct][:, c0 : c0 + csz],
                                start=(ct == 0),
                                stop=(ct == CT - 1),
                            )
                    for ci, (c0, csz) in enumerate(QCH):
                        nc.vector.scalar_tensor_tensor(
                            dst[:, c0 : c0 + csz],
                            pss[ci][:, :csz],
                            1.0,
                            bqk_sb[:, ft : ft + 1].to_broadcast([P, csz]),
                            ALU.mult,
                            ALU.add,
                        )
                return qt, ktb

            def emit_scores(qt, ktb):
                """scores -> exp -> mask for both heads of the pair.
                Returns es0[hi] (kt0, dense) and esg[hi][grp] (packed
                windows) bf16 tiles."""
                es0 = {}
                esg = {0: [None, None], 1: [None, None]}
                mm = nc.tensor.matmul
                for hi in (0, 1):
                    es0[hi] = esp.tile([P, 512], BF16, tag="es0", name="es0")
                # kt0 vs q [0, 512): fills one psum bank per head
                for hi in (0, 1):
                    po = D * hi
                    sc = psc.tile([P, 512], F32, tag="sc", name="sc")
                    mm(
                        sc[:, :512],
                        ktb[po : po + D, 0:128],
                        qt[po : po + D, 0:512],
                        start=True,
                        stop=True,
                    )
                    nc.scalar.activation(es0[hi][:, :512], sc[:, :512], AF.Exp)
                    eng = nc.vector if hi == 0 else nc.gpsimd
                    eng.tensor_tensor(
                        es0[hi][:, 0:512], es0[hi][:, 0:512], mask0_sb[:, 0:512],
                        ALU.mult,
                    )
                # kt1..4 banded windows (2 per psum tile) + kt0 vs q [512, 578)
                # packed into the grp-0 tile
                for grp in (0, 1):
                    wins = WIN[2 * grp : 2 * grp + 2]
                    for hi in (0, 1):
                        po = D * hi
                        sc = psc.tile([P, 512], F32, tag="sc", name="sc")
                        for g, off, a, blen, k0, ksz in wins:
                            mm(
                                sc[:ksz, off + 2 : off + 2 + blen],
                                ktb[po : po + D, k0 : k0 + ksz],
                                qt[po : po + D, a : a + blen],
                                start=True, stop=True, skip_group_check=True,
                            )
                            mm(
                                sc[:ksz, off : off + 2],
                                ktb[po : po + D, k0 : k0 + ksz],
                                qt[po : po + D, 0:2],
                                start=True, stop=True, skip_group_check=True,
                            )
                        if grp == 0:
                            mm(
                                sc[:, K0B : K0B + 66],
                                ktb[po : po + D, 0:128],
                                qt[po : po + D, 512:NP],
                                start=True, stop=True, skip_group_check=True,
                            )
                        gw = GW[grp]
                        es = eswp.tile([P, GW[0]], BF16, tag=f"esg{grp}",
                                       name=f"esg{grp}")
                        esg[hi][grp] = es
                        nc.scalar.activation(es[:, :gw], sc[:, :gw], AF.Exp)
                        eng = nc.vector if hi == 0 else nc.gpsimd
                        m0 = grp * GW[0]
                        eng.tensor_tensor(
                            es[:, :gw], es[:, :gw],
                            maskw_sb[:, m0 : m0 + gw], ALU.mult,
                        )
                return es0, esg

            def emit_av(hp, hi, es0, esg, v_tok, oc_sb, srs):
                """banded attn@v for head h, stage output + denominator."""
                h = 2 * hp + hi
                oe = poe.tile([D + 1, NP + 2], F32, tag="oe", name="oe")
                mm = nc.tensor.matmul
                # kt0 dense: first writer of both PSUM banks (start=True)
                mm(oe[:, 0:512], v_tok[0][:, h, :], es0[hi][:, 0:512],
                   start=True, stop=False, skip_group_check=True)
                mm(oe[:, 512:NP], v_tok[0][:, h, :],
                   esg[hi][0][:, K0B : K0B + 66],
                   start=True, stop=False, skip_group_check=True)
                # banded tiles: accumulate segments (split at bank boundary)
                for wi, (grp, off, a, blen, k0, ksz) in enumerate(WIN):
                    es = esg[hi][grp]
                    vkt = v_tok[wi + 1][:ksz, h, :]
                    s0 = off + 2
                    if a + blen <= 512:
                        segs = [(s0, s0 + blen, a)]
                    else:
                        sp = s0 + (512 - a)
                        segs = [(s0, sp, a), (sp, s0 + blen, 512)]
                    for g0, g1, o0 in segs:
                        mm(oe[:, o0 : o0 + (g1 - g0)], vkt, es[:ksz, g0:g1],
                           start=False, stop=False, skip_group_check=True)
                    # CLS query column accumulates into output column 0
                    # (column 1 adds masked zeros)
                    mm(oe[:, 0:2], vkt, es[:ksz, off : off + 2],
                       start=False, stop=(wi == len(WIN) - 1),
                       skip_group_check=True)
                # head output rows -> oc (hi=1 shifts partitions via DMA);
                # extract fp32 denominator row
                if hi == 0:
                    nc.vector.tensor_copy(oc_sb[hp][0:D, 0:N], oe[0:D, 0:N])
                else:
                    stage = rcpp.tile([D, NP], BF16, tag="stage")
                    nc.vector.tensor_copy(stage[:, 0:N], oe[0:D, 0:N])
                    nc.sync.dma_start(oc_sb[hp][D : 2 * D, 0:N], stage[:, 0:N])
                srf = rcpp.tile([D + 1, NP], F32, tag="srf")
                nc.scalar.copy(srf[D : D + 1, 0:N], oe[D : D + 1, 0:N])
                nc.sync.dma_start(srs[hi : hi + 1, 0:N], srf[D : D + 1, 0:N])

            def emit_recip(srs):
                """reciprocal of the pair's denominators + broadcast DMA."""
                rr = rcpp.tile([2, NP], F32, tag="rr")
                nc.vector.reciprocal_approx_fast(rr[:, 0:N], srs[:, 0:N])
                rrd = drp.tile([2, NP], F32, tag="rrd")
                nc.sync.dma_start(rrd[:, :], rr[:, :])
                rb = rcpp.tile([P, NP], F32, tag="rb")
                nc.sync.dma_start(rb[0:D, 0:N], _bcast_ap(rrd[0][0:N], D))
                nc.sync.dma_start(rb[D : 2 * D, 0:N], _bcast_ap(rrd[1][0:N], D))
                return rb

            def emit_nmul(hp, rb, oc_sb):
                oc = oc_sb[hp]
                for po in (0, D):
                    nc.vector.tensor_tensor(
                        oc[po : po + D, 0:N],
                        oc[po : po + D, 0:N],
                        rb[po : po + D, 0:N],
                        ALU.mult,
                    )

            def emit_proj(b, oc_sb):
                for mt, (m0, msz) in enumerate(KT):
                    ysb = ysp.tile([P, C], F32, tag="ysb", name="ysb")
                    pss = [
                        pmm.tile([P, 512], F32, tag="pb", name="ps")
                        for _ in range(2)
                    ]
                    for ct in range(CT):
                        for ci, (c0, csz) in enumerate(VCH):
                            nc.tensor.matmul(
                                pss[ci][:msz, :csz],
                                oc_sb[ct][:, m0 : m0 + msz],
                                wp_sb[ct][:, c0 : c0 + csz],
                                start=(ct == 0),
                                stop=(ct == CT - 1),
                            )
                    for ci, (c0, csz) in enumerate(VCH):
                        nc.vector.tensor_tensor(
                            ysb[:msz, c0 : c0 + csz],
                            pss[ci][:msz, :csz],
                            bp_sb[:msz, c0 : c0 + csz],
                            ALU.add,
                        )
                    nc.sync.dma_start(
                        y[b * N + m0 : b * N + m0 + msz, :], ysb[:msz, :]
                    )

            # ---- main schedule (software-pipelined across pairs/batches) ----
            v_tok = emit_v(xT_b)
            # cover the wqk DMA wait and warm the clock for the first qk
            for i in range(28):
                ps = pmm.tile([P, 512], F32, tag="pb", name="ps")
                nc.tensor.matmul(ps[:, :512], wup[:, 0:P], wup[:, 0:512],
                                 start=True, stop=True, skip_group_check=True)
            for b in range(BPC):
                oc_sb = [
                    ocp.tile([P, NP], BF16, tag=f"oc{ct}", name=f"oc{ct}")
                    for ct in range(CT)
                ]
                srs_l = []
                rb_l = []
                qk_t = emit_qk(0, xT_b)
                nxt_x = None
                for hp in range(H // 2):
                    qt, ktb = qk_t
                    es0, esg = emit_scores(qt, ktb)
                    if hp == 2 and b + 1 < BPC:
                        nxt_x = emit_x_dma(b + 1)
                    if hp + 1 < H // 2:
                        qk_t = emit_qk(hp + 1, xT_b)
                    srs = rcpp.tile([2, NP], F32, tag="srs")
                    srs_l.append(srs)
                    if hp >= 1:
                        rb_l.append(emit_recip(srs_l[hp - 1]))
                    for hi in (0, 1):
                        emit_av(hp, hi, es0, esg, v_tok, oc_sb, srs)
                    if hp >= 1:
                        emit_nmul(hp - 1, rb_l[hp - 1], oc_sb)
                prev_oc = oc_sb
                rb = emit_recip(srs_l[H // 2 - 1])
                if b + 1 < BPC:
                    xT_b = nxt_x
                    v_tok = emit_v(xT_b)
                emit_nmul(H // 2 - 1, rb, prev_oc)
                emit_proj(b, prev_oc)

    nc.finalize()
    return nc


_CACHE = {}


def _make_in_maps(x, qkv_w, qkv_b, proj_w, proj_b):
    bf = ml_dtypes.bfloat16
    x = np.asarray(x, np.float32)
    qkv_w = np.asarray(qkv_w, np.float32)
    qkv_b = np.asarray(qkv_b, np.float32)
    proj_w = np.asarray(proj_w, np.float32)
    proj_b = np.asarray(proj_b, np.float32)

    wqkT = np.ascontiguousarray(qkv_w[: 2 * C].T).copy()
    wqkT[:, :C] *= SCALE
    wqkT = wqkT.astype(bf)
    wvT = np.ascontiguousarray(qkv_w[2 * C :].T).astype(bf)
    wpT = np.ascontiguousarray(proj_w.T).astype(bf)
    bqk_h = qkv_b[: 2 * C].copy()
    bqk_h[:C] *= SCALE
    bv_h = np.ascontiguousarray(qkv_b[2 * C :])

    m = np.zeros((NP, NP), np.float32)
    m[:N, :N] = _build_mask_np()
    mask0 = m[:P, :].astype(bf)
    maskw = np.zeros((P, MW), np.float32)
    for grp, off, a, blen, k0, ksz in WIN:
        base = grp * GW[0] + off
        maskw[:ksz, base] = 1.0
        maskw[:ksz, base + 1] = m[k0 : k0 + ksz, 1]
        maskw[:ksz, base + 2 : base + 2 + blen] = m[k0 : k0 + ksz, a : a + blen]
    maskw[:, K0B : K0B + 66] = m[:P, 512:NP]
    maskw = maskw.astype(bf)

    in_maps = []
    for c in range(NCORES):
        xp_c = np.zeros((BPC, NP, C), np.float32)
        xp_c[:, :N, :] = x[c * BPC : (c + 1) * BPC]
        xT_c = np.ascontiguousarray(xp_c.reshape(TP, C).T).astype(bf)
        in_maps.append(
            {
                "xT": xT_c,
                "wqkT": wqkT,
                "wvT": wvT,
                "wpT": wpT,
                "bqk": bqk_h,
                "bv": bv_h,
                "bp": proj_b,
                "mask0d": mask0,
                "maskwd": maskw,
                "ones12": np.ones(H, bf),
            }
        )
    return in_maps


def kernel(x, qkv_w, qkv_b, proj_w, proj_b):
    if "nc" not in _CACHE:
        _CACHE["nc"] = _build_program()
    nc = _CACHE["nc"]

    in_maps = _make_in_maps(x, qkv_w, qkv_b, proj_w, proj_b)
    res = run_bass_kernel_spmd(nc, in_maps, list(range(NCORES)))
    out = np.concatenate(
        [res.results[c]["y"].reshape(BPC, N, C) for c in range(NCORES)], axis=0
    )
    return out.astype(np.float32)


# revision 30
# speedup vs baseline: 1.1165x; 1.0068x over previous
"""Trainium2 Bass kernel for CustomAttention (ViT-style windowed attention).

Math (per batch element):
  qkv = x @ qkv_w.T + qkv_b            -> q, k, v  [H=12 heads, D=64]
  s   = (q * D^-0.5) @ k.T             masked by a fixed 24x24-grid window
  attn = softmax(s)                    (CLS row/col always attended)
  out  = attn @ v                      -> concat heads -> @ proj_w.T + proj_b

Sharding: data-parallel over batch across 8 cores (4 images/core).

Key device-side choices:
  - The window mask in row-major token order is a band: patch key j is
    attended only by queries in [j-25, j+25] (plus CLS row/col). Scores and
    attn@v therefore run BANDED per 128-key tile: each key tile streams only
    its ~180-column query window instead of all 578. Key tile 0 keeps the
    full query range (its row 0 is the CLS key, attended by every query).
    The CLS query (attends all keys) lands in column 0 of each window via
    N=1 matmuls; its attn@v contributions accumulate into output column 0.
  - Window score tiles are packed pairwise into one PSUM bank (kt1+kt2,
    kt3+kt4) so exp and mask run once per packed tile.
  - attn@v accumulates banded segments into one [65, 580] PSUM tile spanning
    2 banks; matmuls split at the 512-column bank boundary and the first
    writer of each bank uses start=True (clears has_written for the bank).
  - All matmul operands are bf16 (fp32 PSUM accumulation): bf16 streams
    1 col/cycle at any N (fp32r needs N>=256), enables fast weight load, and
    halves DMA/SBUF. Measured max-rel error ~4e-3 vs the 2e-2 gate.
  - softmax runs unnormalized (no max subtraction; |s| small): exp then
    mask-multiply. v carries an interleaved ones column so attn@v also
    yields the softmax denominators (row 64). Head outputs + denominators
    stage through one bf16 copy; normalization is deferred one pair:
    Scalar-engine reciprocal, DRAM-source partition-broadcast DMA, bf16
    multiply before proj.
  - a burst of junk warmup matmuls at kernel start keeps the PE HAM clock
    gate at full rate while x/weights stream in (x DMAs issued first).
"""

import ml_dtypes
import numpy as np

import concourse.bass as bass
import concourse.mybir as mybir
from concourse import bacc
from concourse.bass_utils import run_bass_kernel_spmd
from concourse.tile import TileContext

B, N, C = 32, 577, 768
H, D = 12, 64
NCORES = 8
BPC = B // NCORES            # batches per core
NP = N + 1                   # padded token count (even)
TP = BPC * NP
T = BPC * N
SCALE = float(D) ** -0.5     # 0.125, exact in bf16
F32 = mybir.dt.float32
BF16 = mybir.dt.bfloat16
P = 128

CT = C // P                                      # 6 contraction tiles
KT = [(0, 128), (128, 128), (256, 128), (384, 128), (512, 65)]
QCH = [(0, 290), (290, 288)]                     # qk / kt0-score chunks
VCH = [(0, 512), (512, 256)]                     # v / proj output chunks
# banded windows for key tiles 1..4: (grp, off, a, blen, k0, ksz)
# grp selects the packed psum/es tile (0: kt1+kt2, 1: kt3+kt4); within it,
# cols off+0/off+1 = scores vs q tokens 0 (CLS) and 1 (masked to zero),
# cols off+2..off+1+blen = q tokens [a, a+blen). All col offsets/sizes even.
WIN = [
    (0, 0, 102, 180, 128, 128),
    (0, 182, 230, 180, 256, 128),
    (1, 0, 358, 180, 384, 128),
    (1, 182, 486, 92, 512, 65),
]
# key tile 0's scores vs q tokens [512, 578) also pack into the grp-0 tile
# at cols [K0B, K0B+66) (its [0, 512) chunk fills a whole bank on its own).
K0B = 364
GW = [430, 276]              # packed window tile widths
MW = sum(GW)                 # banded mask tile width
AF = mybir.ActivationFunctionType
ALU = mybir.AluOpType


def _build_mask_np():
    img = 24
    p = np.arange(img * img)
    pi, pj = p // img, p % img
    ok = (np.abs(pi[:, None] - pi[None, :]) <= 1) & (
        np.abs(pj[:, None] - pj[None, :]) <= 1
    )
    m = np.zeros((N, N), dtype=np.float32)
    m[1:, 1:] = ok
    m[0, :] = True
    m[:, 0] = True
    return m


def _bcast_ap(ap1d, parts):
    """1-row AP -> [parts, n] with partition stride 0 (DRAM-source DMA)."""
    return bass.AP(
        tensor=ap1d.tensor, offset=ap1d.offset, ap=[[0, parts]] + list(ap1d.ap)[-1:]
    )


def _build_program():
    nc = bacc.Bacc("TRN2", target_bir_lowering=False, debug=False)
    xT = nc.dram_tensor("xT", [C, TP], BF16, kind="ExternalInput").ap()
    wqkT = nc.dram_tensor("wqkT", [C, 2 * C], BF16, kind="ExternalInput").ap()
    wvT = nc.dram_tensor("wvT", [C, C], BF16, kind="ExternalInput").ap()
    wpT = nc.dram_tensor("wpT", [C, C], BF16, kind="ExternalInput").ap()
    bqk = nc.dram_tensor("bqk", [2 * C], F32, kind="ExternalInput").ap()
    bv = nc.dram_tensor("bv", [C], F32, kind="ExternalInput").ap()
    bp = nc.dram_tensor("bp", [C], F32, kind="ExternalInput").ap()
    mask0d = nc.dram_tensor("mask0d", [P, NP], BF16, kind="ExternalInput").ap()
    maskwd = nc.dram_tensor("maskwd", [P, MW], BF16, kind="ExternalInput").ap()
    ones12 = nc.dram_tensor("ones12", [H], BF16, kind="ExternalInput").ap()
    y = nc.dram_tensor("y", [T, C], F32, kind="ExternalOutput").ap()

    with TileContext(nc) as tc:
        with (
            tc.tile_pool(name="singles", bufs=1) as singles,
            tc.tile_pool(name="xp", bufs=2) as xp,
            tc.tile_pool(name="qkp", bufs=2) as qkp,
            tc.tile_pool(name="vtp", bufs=2) as vtp,
            tc.tile_pool(name="ocp", bufs=2) as ocp,
            tc.tile_pool(name="esp", bufs=4) as esp,
            tc.tile_pool(name="esw", bufs=8) as eswp,
            tc.tile_pool(name="rcp", bufs=3) as rcpp,
            tc.tile_pool(name="ysp", bufs=2) as ysp,
            tc.tile_pool(name="pmm", bufs=2, space="PSUM") as pmm,
            tc.tile_pool(name="psc", bufs=4, space="PSUM") as psc,
            tc.tile_pool(name="poe", bufs=1, space="PSUM") as poe,
            tc.tile_pool(name="drp", bufs=4, space="DRAM") as drp,
        ):
            # ---- prefetch batch 0's x before the weights ----
            def emit_x_dma(b):
                xT_b = []
                for ct in range(CT):
                    t = xp.tile([P, NP], BF16, tag=f"x{ct}", name=f"x{ct}")
                    nc.sync.dma_start(
                        t[:], xT[ct * P : (ct + 1) * P, b * NP : (b + 1) * NP]
                    )
                    xT_b.append(t)
                return xT_b

            xT_b = emit_x_dma(0)

            # ---- PE warmup: junk matmuls overlap the input DMAs and get
            # the HAM clock gate to 8/8 before real work starts ----
            wup = singles.tile([P, 512], BF16, tag="wup")
            nc.vector.memset(wup[:], 1.0)
            for i in range(24):
                ps = pmm.tile([P, 512], F32, tag="pb", name="ps")
                nc.tensor.matmul(ps[:, :512], wup[:, 0:P], wup[:, 0:512],
                                 start=True, stop=True, skip_group_check=True)

            # ---- persistent loads (v/qk weights first) ----
            wv_sb = []
            wqk_sb = []
            wp_sb = []
            for ct in range(CT):
                t = singles.tile([P, C], BF16, tag=f"wv{ct}")
                nc.sync.dma_start(t[:], wvT[ct * P : (ct + 1) * P, :])
                wv_sb.append(t)
            for ct in range(CT):
                t = singles.tile([P, 2 * C], BF16, tag=f"wqk{ct}")
                nc.sync.dma_start(t[:], wqkT[ct * P : (ct + 1) * P, :])
                wqk_sb.append(t)
            bqk_sb = singles.tile([P, 2 * C // P], F32, tag="bqk")
            nc.sync.dma_start(bqk_sb[:], bqk.rearrange("(o p) -> p o", p=P))
            bv_sb = singles.tile([P, C], F32, tag="bv")
            nc.sync.dma_start(bv_sb[:], _bcast_ap(bv, P))
            ones_sb = singles.tile([P, H], BF16, tag="ones_sb")
            nc.sync.dma_start(ones_sb[:], _bcast_ap(ones12, P))
            mask0_sb = singles.tile([P, NP], BF16, tag="mask0")
            nc.sync.dma_start(mask0_sb[:], mask0d[:, :])
            maskw_sb = singles.tile([P, MW], BF16, tag="maskw")
            nc.sync.dma_start(maskw_sb[:], maskwd[:, :])
            for ct in range(CT):
                t = singles.tile([P, C], BF16, tag=f"wp{ct}")
                nc.sync.dma_start(t[:], wpT[ct * P : (ct + 1) * P, :])
                wp_sb.append(t)
            bp_sb = singles.tile([P, C], F32, tag="bp")
            nc.sync.dma_start(bp_sb[:], _bcast_ap(bp, P))

            def emit_v(xT_b):
                v_tok = []
                for mt, (m0, msz) in enumerate(KT):
                    vt = vtp.tile([P, H, D + 1], BF16, tag=f"vt{mt}", name=f"vt{mt}")
                    nc.vector.tensor_copy(vt[:, :, D : D + 1], ones_sb[:, :, None])
                    pss = [
                        pmm.tile([P, 512], F32, tag="pb", name="ps")
                        for _ in range(2)
                    ]
                    for ct in range(CT):
                        for ci, (c0, csz) in enumerate(VCH):
                            nc.tensor.matmul(
                                pss[ci][:msz, :csz],
                                xT_b[ct][:, m0 : m0 + msz],
                                wv_sb[ct][:, c0 : c0 + csz],
                                start=(ct == 0),
                                stop=(ct == CT - 1),
                            )
                    for ci, (c0, csz) in enumerate(VCH):
                        nh = csz // D
                        h0 = c0 // D
                        nc.vector.tensor_tensor(
                            vt[:msz, h0 : h0 + nh, 0:D],
                            pss[ci][:msz, :csz].rearrange("p (h d) -> p h d", d=D),
                            bv_sb[:msz, c0 : c0 + csz].rearrange(
                                "p (h d) -> p h d", d=D
                            ),
                            ALU.add,
                        )
                    v_tok.append(vt)
                return v_tok

            def emit_qk(hp, xT_b):
                qt = qkp.tile([P, NP], BF16, tag="qk_q")
                ktb = qkp.tile([P, NP], BF16, tag="qk_k")
                for dst, ft in ((qt, hp), (ktb, CT + hp)):
                    pss = [
                        pmm.tile([P, 512], F32, tag="pb", name="ps")
                        for _ in range(2)
                    ]
                    for ct in range(CT):
                        for ci, (c0, csz) in enumerate(QCH):
                            nc.tensor.matmul(
                                pss[ci][:, :csz],
                                wqk_sb[ct][:, ft * P : (ft + 1) * P],
                                xT_b[ct][:, c0 : c0 + csz],
                                start=(ct == 0),
                                stop=(ct == CT - 1),
                            )
                    for ci, (c0, csz) in enumerate(QCH):
                        nc.vector.scalar_tensor_tensor(
                            dst[:, c0 : c0 + csz],
                            pss[ci][:, :csz],
                            1.0,
                            bqk_sb[:, ft : ft + 1].to_broadcast([P, csz]),
                            ALU.mult,
                            ALU.add,
                        )
                return qt, ktb

            def emit_scores(qt, ktb):
                """scores -> exp -> mask for both heads of the pair.
                Returns es0[hi] (kt0, dense) and esg[hi][grp] (packed
                windows) bf16 tiles."""
                es0 = {}
                esg = {0: [None, None], 1: [None, None]}
                mm = nc.tensor.matmul
                for hi in (0, 1):
                    es0[hi] = esp.tile([P, 512], BF16, tag="es0", name="es0")
                # kt0 vs q [0, 512): fills one psum bank per head
                for hi in (0, 1):
                    po = D * hi
                    sc = psc.tile([P, 512], F32, tag="sc", name="sc")
                    mm(
                        sc[:, :512],
                        ktb[po : po + D, 0:128],
                        qt[po : po + D, 0:512],
                        start=True,
                        stop=True,
                    )
                    nc.scalar.activation(es0[hi][:, :512], sc[:, :512], AF.Exp)
                    eng = nc.vector if hi == 0 else nc.gpsimd
                    eng.tensor_tensor(
                        es0[hi][:, 0:512], es0[hi][:, 0:512], mask0_sb[:, 0:512],
                        ALU.mult,
                    )
                # kt1..4 banded windows (2 per psum tile) + kt0 vs q [512, 578)
                # packed into the grp-0 tile
                for grp in (0, 1):
                    wins = WIN[2 * grp : 2 * grp + 2]
                    for hi in (0, 1):
                        po = D * hi
                        sc = psc.tile([P, 512], F32, tag="sc", name="sc")
                        for g, off, a, blen, k0, ksz in wins:
                            mm(
                                sc[:ksz, off + 2 : off + 2 + blen],
                                ktb[po : po + D, k0 : k0 + ksz],
                                qt[po : po + D, a : a + blen],
                                start=True, stop=True, skip_group_check=True,
                            )
                            mm(
                                sc[:ksz, off : off + 2],
                                ktb[po : po + D, k0 : k0 + ksz],
                                qt[po : po + D, 0:2],
                                start=True, stop=True, skip_group_check=True,
                            )
                        if grp == 0:
                            mm(
                                sc[:, K0B : K0B + 66],
                                ktb[po : po + D, 0:128],
                                qt[po : po + D, 512:NP],
                                start=True, stop=True, skip_group_check=True,
                            )
                        gw = GW[grp]
                        es = eswp.tile([P, GW[0]], BF16, tag=f"esg{grp}",
                                       name=f"esg{grp}")
                        esg[hi][grp] = es
                        nc.scalar.activation(es[:, :gw], sc[:, :gw], AF.Exp)
                        eng = nc.vector if hi == 0 else nc.gpsimd
                        m0 = grp * GW[0]
                        eng.tensor_tensor(
                            es[:, :gw], es[:, :gw],
                            maskw_sb[:, m0 : m0 + gw], ALU.mult,
                        )
                return es0, esg

            def emit_av(hp, hi, es0, esg, v_tok, oc_sb, srs):
                """banded attn@v for head h, stage output + denominator."""
                h = 2 * hp + hi
                oe = poe.tile([D + 1, NP + 2], F32, tag="oe", name="oe")
                mm = nc.tensor.matmul
                # kt0 dense: first writer of both PSUM banks (start=True)
                mm(oe[:, 0:512], v_tok[0][:, h, :], es0[hi][:, 0:512],
                   start=True, stop=False, skip_group_check=True)
                mm(oe[:, 512:NP], v_tok[0][:, h, :],
                   esg[hi][0][:, K0B : K0B + 66],
                   start=True, stop=False, skip_group_check=True)
                # banded tiles: accumulate segments (split at bank boundary)
                for wi, (grp, off, a, blen, k0, ksz) in enumerate(WIN):
                    es = esg[hi][grp]
                    vkt = v_tok[wi + 1][:ksz, h, :]
                    s0 = off + 2
                    if a + blen <= 512:
                        segs = [(s0, s0 + blen, a)]
                    else:
                        sp = s0 + (512 - a)
                        segs = [(s0, sp, a), (sp, s0 + blen, 512)]
                    for g0, g1, o0 in segs:
                        mm(oe[:, o0 : o0 + (g1 - g0)], vkt, es[:ksz, g0:g1],
                           start=False, stop=False, skip_group_check=True)
                    # CLS query column accumulates into output column 0
                    # (column 1 adds masked zeros)
                    mm(oe[:, 0:2], vkt, es[:ksz, off : off + 2],
                       start=False, stop=(wi == len(WIN) - 1),
                       skip_group_check=True)
                # head output rows -> oc (hi=1 shifts partitions via DMA);
                # extract fp32 denominator row
                if hi == 0:
                    nc.vector.tensor_copy(oc_sb[hp][0:D, 0:N], oe[0:D, 0:N])
                else:
                    stage = rcpp.tile([D, NP], BF16, tag="stage")
                    nc.vector.tensor_copy(stage[:, 0:N], oe[0:D, 0:N])
                    nc.sync.dma_start(oc_sb[hp][D : 2 * D, 0:N], stage[:, 0:N])
                srf = rcpp.tile([D + 1, NP], F32, tag="srf")
                nc.scalar.copy(srf[D : D + 1, 0:N], oe[D : D + 1, 0:N])
                nc.sync.dma_start(srs[hi : hi + 1, 0:N], srf[D : D + 1, 0:N])

            def emit_recip(srs):
                """reciprocal of the pair's denominators + broadcast DMA."""
                rr = rcpp.tile([2, NP], F32, tag="rr")
                nc.vector.reciprocal_approx_fast(rr[:, 0:N], srs[:, 0:N])
                rrd = drp.tile([2, NP], F32, tag="rrd")
                nc.sync.dma_start(rrd[:, :], rr[:, :])
                rb = rcpp.tile([P, NP], F32, tag="rb")
                nc.sync.dma_start(rb[0:D, 0:N], _bcast_ap(rrd[0][0:N], D))
                nc.sync.dma_start(rb[D : 2 * D, 0:N], _bcast_ap(rrd[1][0:N], D))
                return rb

            def emit_nmul(hp, rb, oc_sb):
                oc = oc_sb[hp]
                for po in (0, D):
                    nc.vector.tensor_tensor(
                        oc[po : po + D, 0:N],
                        oc[po : po + D, 0:N],
                        rb[po : po + D, 0:N],
                        ALU.mult,
                    )

            def emit_proj(b, oc_sb):
                for mt, (m0, msz) in enumerate(KT):
                    ysb = ysp.tile([P, C], F32, tag="ysb", name="ysb")
                    pss = [
                        pmm.tile([P, 512], F32, tag="pb", name="ps")
                        for _ in range(2)
                    ]
                    for ct in range(CT):
                        for ci, (c0, csz) in enumerate(VCH):
                            nc.tensor.matmul(
                                pss[ci][:msz, :csz],
                                oc_sb[ct][:, m0 : m0 + msz],
                                wp_sb[ct][:, c0 : c0 + csz],
                                start=(ct == 0),
                                stop=(ct == CT - 1),
                            )
                    for ci, (c0, csz) in enumerate(VCH):
                        nc.vector.tensor_tensor(
                            ysb[:msz, c0 : c0 + csz],
                            pss[ci][:msz, :csz],
                            bp_sb[:msz, c0 : c0 + csz],
                            ALU.add,
                        )
                    nc.sync.dma_start(
                        y[b * N + m0 : b * N + m0 + msz, :], ysb[:msz, :]
                    )

            # ---- main schedule (software-pipelined across pairs/batches) ----
            v_tok = emit_v(xT_b)
            # cover the wqk DMA wait and warm the clock for the first qk
            for i in range(40):
                ps = pmm.tile([P, 512], F32, tag="pb", name="ps")
                nc.tensor.matmul(ps[:, :512], wup[:, 0:P], wup[:, 0:512],
                                 start=True, stop=True, skip_group_check=True)
            for b in range(BPC):
                oc_sb = [
                    ocp.tile([P, NP], BF16, tag=f"oc{ct}", name=f"oc{ct}")
                    for ct in range(CT)
                ]
                srs_l = []
                rb_l = []
                qk_t = emit_qk(0, xT_b)
                nxt_x = None
                for hp in range(H // 2):
                    qt, ktb = qk_t
                    es0, esg = emit_scores(qt, ktb)
                    if hp == 2 and b + 1 < BPC:
                        nxt_x = emit_x_dma(b + 1)
                    if hp + 1 < H // 2:
                        qk_t = emit_qk(hp + 1, xT_b)
                    srs = rcpp.tile([2, NP], F32, tag="srs")
                    srs_l.append(srs)
                    if hp >= 1:
                        rb_l.append(emit_recip(srs_l[hp - 1]))
                    for hi in (0, 1):
                        emit_av(hp, hi, es0, esg, v_tok, oc_sb, srs)
                    if hp >= 1:
                        emit_nmul(hp - 1, rb_l[hp - 1], oc_sb)
                prev_oc = oc_sb
                rb = emit_recip(srs_l[H // 2 - 1])
                if b + 1 < BPC:
                    xT_b = nxt_x
                    v_tok = emit_v(xT_b)
                else:
                    # no next-batch v GEMM to cover the final norm chain:
                    # keep the PE busy/warm with junk matmuls instead
                    for i in range(12):
                        ps = pmm.tile([P, 512], F32, tag="pb", name="ps")
                        nc.tensor.matmul(ps[:, :512], wup[:, 0:P], wup[:, 0:512],
                                         start=True, stop=True,
                                         skip_group_check=True)
                emit_nmul(H // 2 - 1, rb, prev_oc)
                emit_proj(b, prev_oc)

    nc.finalize()
    return nc


_CACHE = {}


def _make_in_maps(x, qkv_w, qkv_b, proj_w, proj_b):
    bf = ml_dtypes.bfloat16
    x = np.asarray(x, np.float32)
    qkv_w = np.asarray(qkv_w, np.float32)
    qkv_b = np.asarray(qkv_b, np.float32)
    proj_w = np.asarray(proj_w, np.float32)
    proj_b = np.asarray(proj_b, np.float32)

    wqkT = np.ascontiguousarray(qkv_w[: 2 * C].T).copy()
    wqkT[:, :C] *= SCALE
    wqkT = wqkT.astype(bf)
    wvT = np.ascontiguousarray(qkv_w[2 * C :].T).astype(bf)
    wpT = np.ascontiguousarray(proj_w.T).astype(bf)
    bqk_h = qkv_b[: 2 * C].copy()
    bqk_h[:C] *= SCALE
    bv_h = np.ascontiguousarray(qkv_b[2 * C :])

    m = np.zeros((NP, NP), np.float32)
    m[:N, :N] = _build_mask_np()
    mask0 = m[:P, :].astype(bf)
    maskw = np.zeros((P, MW), np.float32)
    for grp, off, a, blen, k0, ksz in WIN:
        base = grp * GW[0] + off
        maskw[:ksz, base] = 1.0
        maskw[:ksz, base + 1] = m[k0 : k0 + ksz, 1]
        maskw[:ksz, base + 2 : base + 2 + blen] = m[k0 : k0 + ksz, a : a + blen]
    maskw[:, K0B : K0B + 66] = m[:P, 512:NP]
    maskw = maskw.astype(bf)

    in_maps = []
    for c in range(NCORES):
        xp_c = np.zeros((BPC, NP, C), np.float32)
        xp_c[:, :N, :] = x[c * BPC : (c + 1) * BPC]
        xT_c = np.ascontiguousarray(xp_c.reshape(TP, C).T).astype(bf)
        in_maps.append(
            {
                "xT": xT_c,
                "wqkT": wqkT,
                "wvT": wvT,
                "wpT": wpT,
                "bqk": bqk_h,
                "bv": bv_h,
                "bp": proj_b,
                "mask0d": mask0,
                "maskwd": maskw,
                "ones12": np.ones(H, bf),
            }
        )
    return in_maps


def kernel(x, qkv_w, qkv_b, proj_w, proj_b):
    if "nc" not in _CACHE:
        _CACHE["nc"] = _build_program()
    nc = _CACHE["nc"]

    in_maps = _make_in_maps(x, qkv_w, qkv_b, proj_w, proj_b)
    res = run_bass_kernel_spmd(nc, in_maps, list(range(NCORES)))
    out = np.concatenate(
        [res.results[c]["y"].reshape(BPC, N, C) for c in range(NCORES)], axis=0
    )
    return out.astype(np.float32)


# revision 31
# speedup vs baseline: 1.2852x; 1.1511x over previous
"""Trainium2 Bass kernel for CustomAttention (ViT-style windowed attention).

Math (per batch element):
  qkv = x @ qkv_w.T + qkv_b            -> q, k, v  [H=12 heads, D=64]
  s   = (q * D^-0.5) @ k.T             masked by a fixed 24x24-grid window
  attn = softmax(s)                    (CLS row/col always attended)
  out  = attn @ v                      -> concat heads -> @ proj_w.T + proj_b

Sharding: data-parallel over batch across 8 cores (4 images/core).

Key device-side choices:
  - The window mask in row-major token order is a band: patch key j is
    attended only by queries in [j-25, j+25] (plus CLS row/col). Scores and
    attn@v therefore run BANDED per 128-key tile: each key tile streams only
    its ~180-column query window instead of all 578. Key tile 0 keeps the
    full query range (its row 0 is the CLS key, attended by every query).
    The CLS query (attends all keys) lands in column 0 of each window via
    N=1 matmuls; its attn@v contributions accumulate into output column 0.
  - Window score tiles are packed pairwise into one PSUM bank (kt1+kt2,
    kt3+kt4) so exp and mask run once per packed tile.
  - attn@v accumulates banded segments into one [65, 580] PSUM tile spanning
    2 banks; matmuls split at the 512-column bank boundary and the first
    writer of each bank uses start=True (clears has_written for the bank).
  - All matmul operands are bf16 (fp32 PSUM accumulation): bf16 streams
    1 col/cycle at any N (fp32r needs N>=256), enables fast weight load, and
    halves DMA/SBUF. Measured max-rel error ~4e-3 vs the 2e-2 gate.
  - softmax runs unnormalized (no max subtraction; |s| small): exp then
    mask-multiply. v carries an interleaved ones column so attn@v also
    yields the softmax denominators (row 64). Head outputs + denominators
    stage through one bf16 copy; normalization is deferred one pair:
    Scalar-engine reciprocal, DRAM-source partition-broadcast DMA, bf16
    multiply before proj.
  - a burst of junk warmup matmuls at kernel start keeps the PE HAM clock
    gate at full rate while x/weights stream in (x DMAs issued first).
"""

import ml_dtypes
import numpy as np

import concourse.bass as bass
import concourse.mybir as mybir
from concourse import bacc
from concourse.bass_utils import run_bass_kernel_spmd
from concourse.tile import TileContext

B, N, C = 32, 577, 768
H, D = 12, 64
NCORES = 8
BPC = B // NCORES            # batches per core
NP = N + 1                   # padded token count (even)
TP = BPC * NP
T = BPC * N
SCALE = float(D) ** -0.5     # 0.125, exact in bf16
F32 = mybir.dt.float32
BF16 = mybir.dt.bfloat16
P = 128

CT = C // P                                      # 6 contraction tiles
KT = [(0, 128), (128, 128), (256, 128), (384, 128), (512, 65)]
QCH = [(0, 290), (290, 288)]                     # qk / kt0-score chunks
VCH = [(0, 512), (512, 256)]                     # v / proj output chunks
# banded windows for key tiles 1..4: (grp, off, a, blen, k0, ksz)
# grp selects the packed psum/es tile (0: kt1+kt2, 1: kt3+kt4); within it,
# cols off+0/off+1 = scores vs q tokens 0 (CLS) and 1 (masked to zero),
# cols off+2..off+1+blen = q tokens [a, a+blen). All col offsets/sizes even.
WIN = [
    (0, 0, 102, 180, 128, 128),
    (0, 182, 230, 180, 256, 128),
    (1, 0, 358, 180, 384, 128),
    (1, 182, 486, 92, 512, 65),
]
# key tile 0's scores vs q tokens [512, 578) also pack into the grp-0 tile
# at cols [K0B, K0B+66) (its [0, 512) chunk fills a whole bank on its own).
K0B = 364
GW = [430, 276]              # packed window tile widths
MW = sum(GW)                 # banded mask tile width
AF = mybir.ActivationFunctionType
ALU = mybir.AluOpType


def _build_mask_np():
    img = 24
    p = np.arange(img * img)
    pi, pj = p // img, p % img
    ok = (np.abs(pi[:, None] - pi[None, :]) <= 1) & (
        np.abs(pj[:, None] - pj[None, :]) <= 1
    )
    m = np.zeros((N, N), dtype=np.float32)
    m[1:, 1:] = ok
    m[0, :] = True
    m[:, 0] = True
    return m


def _bcast_ap(ap1d, parts):
    """1-row AP -> [parts, n] with partition stride 0 (DRAM-source DMA)."""
    return bass.AP(
        tensor=ap1d.tensor, offset=ap1d.offset, ap=[[0, parts]] + list(ap1d.ap)[-1:]
    )


def _build_program():
    nc = bacc.Bacc("TRN2", target_bir_lowering=False, debug=False)
    xT = nc.dram_tensor("xT", [C, TP], BF16, kind="ExternalInput").ap()
    wqkT = nc.dram_tensor("wqkT", [C, 2 * C], BF16, kind="ExternalInput").ap()
    wvT = nc.dram_tensor("wvT", [C, C], BF16, kind="ExternalInput").ap()
    wpT = nc.dram_tensor("wpT", [C, C], BF16, kind="ExternalInput").ap()
    bqk = nc.dram_tensor("bqk", [2 * C], F32, kind="ExternalInput").ap()
    bv = nc.dram_tensor("bv", [C], F32, kind="ExternalInput").ap()
    bp = nc.dram_tensor("bp", [C], F32, kind="ExternalInput").ap()
    mask0d = nc.dram_tensor("mask0d", [P, NP], BF16, kind="ExternalInput").ap()
    maskwd = nc.dram_tensor("maskwd", [P, MW], BF16, kind="ExternalInput").ap()
    ones12 = nc.dram_tensor("ones12", [H], BF16, kind="ExternalInput").ap()
    y = nc.dram_tensor("y", [T, C], F32, kind="ExternalOutput").ap()

    with TileContext(nc) as tc:
        with (
            tc.tile_pool(name="singles", bufs=1) as singles,
            tc.tile_pool(name="xp", bufs=2) as xp,
            tc.tile_pool(name="qkp", bufs=2) as qkp,
            tc.tile_pool(name="vtp", bufs=2) as vtp,
            tc.tile_pool(name="ocp", bufs=2) as ocp,
            tc.tile_pool(name="esp", bufs=4) as esp,
            tc.tile_pool(name="esw", bufs=8) as eswp,
            tc.tile_pool(name="rcp", bufs=3) as rcpp,
            tc.tile_pool(name="ysp", bufs=2) as ysp,
            tc.tile_pool(name="pmm", bufs=4, space="PSUM") as pmm,
            tc.tile_pool(name="psc", bufs=2, space="PSUM") as psc,
            tc.tile_pool(name="poe", bufs=1, space="PSUM") as poe,
            tc.tile_pool(name="drp", bufs=4, space="DRAM") as drp,
        ):
            # ---- prefetch batch 0's x before the weights ----
            def emit_x_dma(b):
                xT_b = []
                for ct in range(CT):
                    t = xp.tile([P, NP], BF16, tag=f"x{ct}", name=f"x{ct}")
                    nc.sync.dma_start(
                        t[:], xT[ct * P : (ct + 1) * P, b * NP : (b + 1) * NP]
                    )
                    xT_b.append(t)
                return xT_b

            xT_b = emit_x_dma(0)

            # ---- PE warmup: junk matmuls overlap the input DMAs and get
            # the HAM clock gate to 8/8 before real work starts ----
            wup = singles.tile([P, 512], BF16, tag="wup")
            nc.vector.memset(wup[:], 1.0)
            for i in range(24):
                ps = pmm.tile([P, 512], F32, tag="pb", name="ps")
                nc.tensor.matmul(ps[:, :512], wup[:, 0:P], wup[:, 0:512],
                                 start=True, stop=True, skip_group_check=True)

            # ---- persistent loads (v/qk weights first) ----
            wv_sb = []
            wqk_sb = []
            wp_sb = []
            for ct in range(CT):
                t = singles.tile([P, C], BF16, tag=f"wv{ct}")
                nc.sync.dma_start(t[:], wvT[ct * P : (ct + 1) * P, :])
                wv_sb.append(t)
            for ct in range(CT):
                t = singles.tile([P, 2 * C], BF16, tag=f"wqk{ct}")
                nc.sync.dma_start(t[:], wqkT[ct * P : (ct + 1) * P, :])
                wqk_sb.append(t)
            bqk_sb = singles.tile([P, 2 * C // P], F32, tag="bqk")
            nc.sync.dma_start(bqk_sb[:], bqk.rearrange("(o p) -> p o", p=P))
            bv_sb = singles.tile([P, C], F32, tag="bv")
            nc.sync.dma_start(bv_sb[:], _bcast_ap(bv, P))
            ones_sb = singles.tile([P, H], BF16, tag="ones_sb")
            nc.sync.dma_start(ones_sb[:], _bcast_ap(ones12, P))
            mask0_sb = singles.tile([P, NP], BF16, tag="mask0")
            nc.sync.dma_start(mask0_sb[:], mask0d[:, :])
            maskw_sb = singles.tile([P, MW], BF16, tag="maskw")
            nc.sync.dma_start(maskw_sb[:], maskwd[:, :])
            for ct in range(CT):
                t = singles.tile([P, C], BF16, tag=f"wp{ct}")
                nc.sync.dma_start(t[:], wpT[ct * P : (ct + 1) * P, :])
                wp_sb.append(t)
            bp_sb = singles.tile([P, C], F32, tag="bp")
            nc.sync.dma_start(bp_sb[:], _bcast_ap(bp, P))

            def emit_v(xT_b):
                v_tok = []
                for mt, (m0, msz) in enumerate(KT):
                    vt = vtp.tile([P, H, D + 1], BF16, tag=f"vt{mt}", name=f"vt{mt}")
                    nc.vector.tensor_copy(vt[:, :, D : D + 1], ones_sb[:, :, None])
                    pss = [
                        pmm.tile([P, 512], F32, tag="pb", name="ps")
                        for _ in range(2)
                    ]
                    for ct in range(CT):
                        for ci, (c0, csz) in enumerate(VCH):
                            nc.tensor.matmul(
                                pss[ci][:msz, :csz],
                                xT_b[ct][:, m0 : m0 + msz],
                                wv_sb[ct][:, c0 : c0 + csz],
                                start=(ct == 0),
                                stop=(ct == CT - 1),
                            )
                    for ci, (c0, csz) in enumerate(VCH):
                        nh = csz // D
                        h0 = c0 // D
                        nc.vector.tensor_tensor(
                            vt[:msz, h0 : h0 + nh, 0:D],
                            pss[ci][:msz, :csz].rearrange("p (h d) -> p h d", d=D),
                            bv_sb[:msz, c0 : c0 + csz].rearrange(
                                "p (h d) -> p h d", d=D
                            ),
                            ALU.add,
                        )
                    v_tok.append(vt)
                return v_tok

            def emit_qk(hp, xT_b):
                qt = qkp.tile([P, NP], BF16, tag="qk_q")
                ktb = qkp.tile([P, NP], BF16, tag="qk_k")
                for dst, ft in ((qt, hp), (ktb, CT + hp)):
                    pss = [
                        pmm.tile([P, 512], F32, tag="pb", name="ps")
                        for _ in range(2)
                    ]
                    for ct in range(CT):
                        for ci, (c0, csz) in enumerate(QCH):
                            nc.tensor.matmul(
                                pss[ci][:, :csz],
                                wqk_sb[ct][:, ft * P : (ft + 1) * P],
                                xT_b[ct][:, c0 : c0 + csz],
                                start=(ct == 0),
                                stop=(ct == CT - 1),
                            )
                    for ci, (c0, csz) in enumerate(QCH):
                        nc.vector.scalar_tensor_tensor(
                            dst[:, c0 : c0 + csz],
                            pss[ci][:, :csz],
                            1.0,
                            bqk_sb[:, ft : ft + 1].to_broadcast([P, csz]),
                            ALU.mult,
                            ALU.add,
                        )
                return qt, ktb

            def emit_scores(qt, ktb):
                """scores -> exp -> mask for both heads of the pair.
                Returns es0[hi] (kt0, dense) and esg[hi][grp] (packed
                windows) bf16 tiles."""
                es0 = {}
                esg = {0: [None, None], 1: [None, None]}
                mm = nc.tensor.matmul
                for hi in (0, 1):
                    es0[hi] = esp.tile([P, 512], BF16, tag="es0", name="es0")
                # kt0 vs q [0, 512): fills one psum bank per head
                for hi in (0, 1):
                    po = D * hi
                    sc = psc.tile([P, 512], F32, tag="sc", name="sc")
                    mm(
                        sc[:, :512],
                        ktb[po : po + D, 0:128],
                        qt[po : po + D, 0:512],
                        start=True,
                        stop=True,
                    )
                    nc.scalar.activation(es0[hi][:, :512], sc[:, :512], AF.Exp)
                    eng = nc.vector if hi == 0 else nc.gpsimd
                    eng.tensor_tensor(
                        es0[hi][:, 0:512], es0[hi][:, 0:512], mask0_sb[:, 0:512],
                        ALU.mult,
                    )
                # kt1..4 banded windows (2 per psum tile) + kt0 vs q [512, 578)
                # packed into the grp-0 tile
                for grp in (0, 1):
                    wins = WIN[2 * grp : 2 * grp + 2]
                    for hi in (0, 1):
                        po = D * hi
                        sc = psc.tile([P, 512], F32, tag="sc", name="sc")
                        for g, off, a, blen, k0, ksz in wins:
                            mm(
                                sc[:ksz, off + 2 : off + 2 + blen],
                                ktb[po : po + D, k0 : k0 + ksz],
                                qt[po : po + D, a : a + blen],
                                start=True, stop=True, skip_group_check=True,
                            )
                            mm(
                                sc[:ksz, off : off + 2],
                                ktb[po : po + D, k0 : k0 + ksz],
                                qt[po : po + D, 0:2],
                                start=True, stop=True, skip_group_check=True,
                            )
                        if grp == 0:
                            mm(
                                sc[:, K0B : K0B + 66],
                                ktb[po : po + D, 0:128],
                                qt[po : po + D, 512:NP],
                                start=True, stop=True, skip_group_check=True,
                            )
                        gw = GW[grp]
                        es = eswp.tile([P, GW[0]], BF16, tag=f"esg{grp}",
                                       name=f"esg{grp}")
                        esg[hi][grp] = es
                        nc.scalar.activation(es[:, :gw], sc[:, :gw], AF.Exp)
                        eng = nc.vector if hi == 0 else nc.gpsimd
                        m0 = grp * GW[0]
                        eng.tensor_tensor(
                            es[:, :gw], es[:, :gw],
                            maskw_sb[:, m0 : m0 + gw], ALU.mult,
                        )
                return es0, esg

            def emit_av(hp, hi, es0, esg, v_tok, oc_sb, srs):
                """banded attn@v for head h, stage output + denominator."""
                h = 2 * hp + hi
                oe = poe.tile([D + 1, NP + 2], F32, tag="oe", name="oe")
                mm = nc.tensor.matmul
                # kt0 dense: first writer of both PSUM banks (start=True)
                mm(oe[:, 0:512], v_tok[0][:, h, :], es0[hi][:, 0:512],
                   start=True, stop=False, skip_group_check=True)
                mm(oe[:, 512:NP], v_tok[0][:, h, :],
                   esg[hi][0][:, K0B : K0B + 66],
                   start=True, stop=False, skip_group_check=True)
                # banded tiles: accumulate segments (split at bank boundary)
                for wi, (grp, off, a, blen, k0, ksz) in enumerate(WIN):
                    es = esg[hi][grp]
                    vkt = v_tok[wi + 1][:ksz, h, :]
                    s0 = off + 2
                    if a + blen <= 512:
                        segs = [(s0, s0 + blen, a)]
                    else:
                        sp = s0 + (512 - a)
                        segs = [(s0, sp, a), (sp, s0 + blen, 512)]
                    for g0, g1, o0 in segs:
                        mm(oe[:, o0 : o0 + (g1 - g0)], vkt, es[:ksz, g0:g1],
                           start=False, stop=False, skip_group_check=True)
                    # CLS query column accumulates into output column 0
                    # (column 1 adds masked zeros)
                    mm(oe[:, 0:2], vkt, es[:ksz, off : off + 2],
                       start=False, stop=(wi == len(WIN) - 1),
                       skip_group_check=True)
                # head output rows -> oc (hi=1 shifts partitions via DMA);
                # extract fp32 denominator row
                if hi == 0:
                    nc.vector.tensor_copy(oc_sb[hp][0:D, 0:N], oe[0:D, 0:N])
                else:
                    stage = rcpp.tile([D, NP], BF16, tag="stage")
                    nc.vector.tensor_copy(stage[:, 0:N], oe[0:D, 0:N])
                    nc.sync.dma_start(oc_sb[hp][D : 2 * D, 0:N], stage[:, 0:N])
                srf = rcpp.tile([D + 1, NP], F32, tag="srf")
                nc.scalar.copy(srf[D : D + 1, 0:N], oe[D : D + 1, 0:N])
                nc.sync.dma_start(srs[hi : hi + 1, 0:N], srf[D : D + 1, 0:N])

            def emit_recip(srs):
                """reciprocal of the pair's denominators + broadcast DMA."""
                rr = rcpp.tile([2, NP], F32, tag="rr")
                nc.vector.reciprocal_approx_fast(rr[:, 0:N], srs[:, 0:N])
                rrd = drp.tile([2, NP], F32, tag="rrd")
                nc.sync.dma_start(rrd[:, :], rr[:, :])
                rb = rcpp.tile([P, NP], F32, tag="rb")
                nc.sync.dma_start(rb[0:D, 0:N], _bcast_ap(rrd[0][0:N], D))
                nc.sync.dma_start(rb[D : 2 * D, 0:N], _bcast_ap(rrd[1][0:N], D))
                return rb

            def emit_nmul(hp, rb, oc_sb):
                oc = oc_sb[hp]
                for po in (0, D):
                    nc.vector.tensor_tensor(
                        oc[po : po + D, 0:N],
                        oc[po : po + D, 0:N],
                        rb[po : po + D, 0:N],
                        ALU.mult,
                    )

            def emit_proj(b, oc_sb):
                for mt, (m0, msz) in enumerate(KT):
                    ysb = ysp.tile([P, C], F32, tag="ysb", name="ysb")
                    pss = [
                        pmm.tile([P, 512], F32, tag="pb", name="ps")
                        for _ in range(2)
                    ]
                    for ct in range(CT):
                        for ci, (c0, csz) in enumerate(VCH):
                            nc.tensor.matmul(
                                pss[ci][:msz, :csz],
                                oc_sb[ct][:, m0 : m0 + msz],
                                wp_sb[ct][:, c0 : c0 + csz],
                                start=(ct == 0),
                                stop=(ct == CT - 1),
                            )
                    for ci, (c0, csz) in enumerate(VCH):
                        nc.vector.tensor_tensor(
                            ysb[:msz, c0 : c0 + csz],
                            pss[ci][:msz, :csz],
                            bp_sb[:msz, c0 : c0 + csz],
                            ALU.add,
                        )
                    nc.sync.dma_start(
                        y[b * N + m0 : b * N + m0 + msz, :], ysb[:msz, :]
                    )

            # ---- main schedule (software-pipelined across pairs/batches) ----
            v_tok = emit_v(xT_b)
            # cover the wqk DMA wait and warm the clock for the first qk
            for i in range(40):
                ps = pmm.tile([P, 512], F32, tag="pb", name="ps")
                nc.tensor.matmul(ps[:, :512], wup[:, 0:P], wup[:, 0:512],
                                 start=True, stop=True, skip_group_check=True)
            for b in range(BPC):
                oc_sb = [
                    ocp.tile([P, NP], BF16, tag=f"oc{ct}", name=f"oc{ct}")
                    for ct in range(CT)
                ]
                srs_l = []
                rb_l = []
                qk_t = emit_qk(0, xT_b)
                nxt_x = None
                for hp in range(H // 2):
                    qt, ktb = qk_t
                    es0, esg = emit_scores(qt, ktb)
                    if hp == 2 and b + 1 < BPC:
                        nxt_x = emit_x_dma(b + 1)
                    if hp + 1 < H // 2:
                        qk_t = emit_qk(hp + 1, xT_b)
                    srs = rcpp.tile([2, NP], F32, tag="srs")
                    srs_l.append(srs)
                    if hp >= 1:
                        rb_l.append(emit_recip(srs_l[hp - 1]))
                    for hi in (0, 1):
                        emit_av(hp, hi, es0, esg, v_tok, oc_sb, srs)
                    if hp >= 1:
                        emit_nmul(hp - 1, rb_l[hp - 1], oc_sb)
                prev_oc = oc_sb
                rb = emit_recip(srs_l[H // 2 - 1])
                if b + 1 < BPC:
                    xT_b = nxt_x
                    v_tok = emit_v(xT_b)
                else:
                    # no next-batch v GEMM to cover the final norm chain:
                    # keep the PE busy/warm with junk matmuls instead
                    for i in range(12):
                        ps = pmm.tile([P, 512], F32, tag="pb", name="ps")
                        nc.tensor.matmul(ps[:, :512], wup[:, 0:P], wup[:, 0:512],
                                         start=True, stop=True,
                                         skip_group_check=True)
                emit_nmul(H // 2 - 1, rb, prev_oc)
                emit_proj(b, prev_oc)

    nc.finalize()
    return nc


_CACHE = {}


def _make_in_maps(x, qkv_w, qkv_b, proj_w, proj_b):
    bf = ml_dtypes.bfloat16
    x = np.asarray(x, np.float32)
    qkv_w = np.asarray(qkv_w, np.float32)
    qkv_b = np.asarray(qkv_b, np.float32)
    proj_w = np.asarray(proj_w, np.float32)
    proj_b = np.asarray(proj_b, np.float32)

    wqkT = np.ascontiguousarray(qkv_w[: 2 * C].T).copy()
    wqkT[:, :C] *= SCALE
    wqkT = wqkT.astype(bf)
    wvT = np.ascontiguousarray(qkv_w[2 * C :].T).astype(bf)
    wpT = np.ascontiguousarray(proj_w.T).astype(bf)
    bqk_h = qkv_b[: 2 * C].copy()
    bqk_h[:C] *= SCALE
    bv_h = np.ascontiguousarray(qkv_b[2 * C :])

    m = np.zeros((NP, NP), np.float32)
    m[:N, :N] = _build_mask_np()
    mask0 = m[:P, :].astype(bf)
    maskw = np.zeros((P, MW), np.float32)
    for grp, off, a, blen, k0, ksz in WIN:
        base = grp * GW[0] + off
        maskw[:ksz, base] = 1.0
        maskw[:ksz, base + 1] = m[k0 : k0 + ksz, 1]
        maskw[:ksz, base + 2 : base + 2 + blen] = m[k0 : k0 + ksz, a : a + blen]
    maskw[:, K0B : K0B + 66] = m[:P, 512:NP]
    maskw = maskw.astype(bf)

    in_maps = []
    for c in range(NCORES):
        xp_c = np.zeros((BPC, NP, C), np.float32)
        xp_c[:, :N, :] = x[c * BPC : (c + 1) * BPC]
        xT_c = np.ascontiguousarray(xp_c.reshape(TP, C).T).astype(bf)
        in_maps.append(
            {
                "xT": xT_c,
                "wqkT": wqkT,
                "wvT": wvT,
                "wpT": wpT,
                "bqk": bqk_h,
                "bv": bv_h,
                "bp": proj_b,
                "mask0d": mask0,
                "maskwd": maskw,
                "ones12": np.ones(H, bf),
            }
        )
    return in_maps


def kernel(x, qkv_w, qkv_b, proj_w, proj_b):
    if "nc" not in _CACHE:
        _CACHE["nc"] = _build_program()
    nc = _CACHE["nc"]

    in_maps = _make_in_maps(x, qkv_w, qkv_b, proj_w, proj_b)
    res = run_bass_kernel_spmd(nc, in_maps, list(range(NCORES)))
    out = np.concatenate(
        [res.results[c]["y"].reshape(BPC, N, C) for c in range(NCORES)], axis=0
    )
    return out.astype(np.float32)


# revision 32
# speedup vs baseline: 1.3216x; 1.0283x over previous
"""Trainium2 Bass kernel for CustomAttention (ViT-style windowed attention).

Math (per batch element):
  qkv = x @ qkv_w.T + qkv_b            -> q, k, v  [H=12 heads, D=64]
  s   = (q * D^-0.5) @ k.T             masked by a fixed 24x24-grid window
  attn = softmax(s)                    (CLS row/col always attended)
  out  = attn @ v                      -> concat heads -> @ proj_w.T + proj_b

Sharding: data-parallel over batch across 8 cores (4 images/core).

Key device-side choices:
  - The window mask in row-major token order is a band: patch key j is
    attended only by queries in [j-25, j+25] (plus CLS row/col). Scores and
    attn@v therefore run BANDED per 128-key tile: each key tile streams only
    its ~180-column query window instead of all 578. Key tile 0 keeps the
    full query range (its row 0 is the CLS key, attended by every query).
    The CLS query (attends all keys) lands in column 0 of each window via
    N=1 matmuls; its attn@v contributions accumulate into output column 0.
  - Window score tiles are packed pairwise into one PSUM bank (kt1+kt2,
    kt3+kt4) so exp and mask run once per packed tile.
  - attn@v accumulates banded segments into one [65, 580] PSUM tile spanning
    2 banks; matmuls split at the 512-column bank boundary and the first
    writer of each bank uses start=True (clears has_written for the bank).
  - All matmul operands are bf16 (fp32 PSUM accumulation): bf16 streams
    1 col/cycle at any N (fp32r needs N>=256), enables fast weight load, and
    halves DMA/SBUF. Measured max-rel error ~4e-3 vs the 2e-2 gate.
  - softmax runs unnormalized (no max subtraction; |s| small): exp then
    mask-multiply. v carries an interleaved ones column so attn@v also
    yields the softmax denominators (row 64). Head outputs + denominators
    stage through one bf16 copy; normalization is deferred one pair:
    Scalar-engine reciprocal, DRAM-source partition-broadcast DMA, bf16
    multiply before proj.
  - a burst of junk warmup matmuls at kernel start keeps the PE HAM clock
    gate at full rate while x/weights stream in (x DMAs issued first).
"""

import ml_dtypes
import numpy as np

import concourse.bass as bass
import concourse.mybir as mybir
from concourse import bacc
from concourse.bass_utils import run_bass_kernel_spmd
from concourse.tile import TileContext

B, N, C = 32, 577, 768
H, D = 12, 64
NCORES = 8
BPC = B // NCORES            # batches per core
NP = N + 1                   # padded token count (even)
TP = BPC * NP
T = BPC * N
SCALE = float(D) ** -0.5     # 0.125, exact in bf16
F32 = mybir.dt.float32
BF16 = mybir.dt.bfloat16
P = 128

CT = C // P                                      # 6 contraction tiles
KT = [(0, 128), (128, 128), (256, 128), (384, 128), (512, 65)]
QCH = [(0, 290), (290, 288)]                     # qk / kt0-score chunks
VCH = [(0, 512), (512, 256)]                     # v / proj output chunks
# banded windows for key tiles 1..4: (grp, off, a, blen, k0, ksz)
# grp selects the packed psum/es tile (0: kt1+kt2, 1: kt3+kt4); within it,
# cols off+0/off+1 = scores vs q tokens 0 (CLS) and 1 (masked to zero),
# cols off+2..off+1+blen = q tokens [a, a+blen). All col offsets/sizes even.
WIN = [
    (0, 0, 102, 180, 128, 128),
    (0, 182, 230, 180, 256, 128),
    (1, 0, 358, 180, 384, 128),
    (1, 182, 486, 92, 512, 65),
]
# key tile 0's scores vs q tokens [512, 578) also pack into the grp-0 tile
# at cols [K0B, K0B+66) (its [0, 512) chunk fills a whole bank on its own).
K0B = 364
GW = [430, 276]              # packed window tile widths
MW = sum(GW)                 # banded mask tile width
AF = mybir.ActivationFunctionType
ALU = mybir.AluOpType


def _build_mask_np():
    img = 24
    p = np.arange(img * img)
    pi, pj = p // img, p % img
    ok = (np.abs(pi[:, None] - pi[None, :]) <= 1) & (
        np.abs(pj[:, None] - pj[None, :]) <= 1
    )
    m = np.zeros((N, N), dtype=np.float32)
    m[1:, 1:] = ok
    m[0, :] = True
    m[:, 0] = True
    return m


def _bcast_ap(ap1d, parts):
    """1-row AP -> [parts, n] with partition stride 0 (DRAM-source DMA)."""
    return bass.AP(
        tensor=ap1d.tensor, offset=ap1d.offset, ap=[[0, parts]] + list(ap1d.ap)[-1:]
    )


def _build_program():
    nc = bacc.Bacc("TRN2", target_bir_lowering=False, debug=False)
    xT = nc.dram_tensor("xT", [C, TP], BF16, kind="ExternalInput").ap()
    wqkT = nc.dram_tensor("wqkT", [C, 2 * C], BF16, kind="ExternalInput").ap()
    wvT = nc.dram_tensor("wvT", [C, C], BF16, kind="ExternalInput").ap()
    wpT = nc.dram_tensor("wpT", [C, C], BF16, kind="ExternalInput").ap()
    bqk = nc.dram_tensor("bqk", [2 * C], F32, kind="ExternalInput").ap()
    bv = nc.dram_tensor("bv", [C], F32, kind="ExternalInput").ap()
    bp = nc.dram_tensor("bp", [C], F32, kind="ExternalInput").ap()
    mask0d = nc.dram_tensor("mask0d", [P, NP], BF16, kind="ExternalInput").ap()
    maskwd = nc.dram_tensor("maskwd", [P, MW], BF16, kind="ExternalInput").ap()
    ones12 = nc.dram_tensor("ones12", [H], BF16, kind="ExternalInput").ap()
    y = nc.dram_tensor("y", [T, C], F32, kind="ExternalOutput").ap()

    with TileContext(nc) as tc:
        with (
            tc.tile_pool(name="singles", bufs=1) as singles,
            tc.tile_pool(name="xp", bufs=2) as xp,
            tc.tile_pool(name="qkp", bufs=2) as qkp,
            tc.tile_pool(name="vtp", bufs=2) as vtp,
            tc.tile_pool(name="ocp", bufs=2) as ocp,
            tc.tile_pool(name="esp", bufs=4) as esp,
            tc.tile_pool(name="esw", bufs=8) as eswp,
            tc.tile_pool(name="rcp", bufs=3) as rcpp,
            tc.tile_pool(name="ysp", bufs=2) as ysp,
            tc.tile_pool(name="pmm", bufs=4, space="PSUM") as pmm,
            tc.tile_pool(name="psc", bufs=2, space="PSUM") as psc,
            tc.tile_pool(name="poe", bufs=1, space="PSUM") as poe,
            tc.tile_pool(name="drp", bufs=4, space="DRAM") as drp,
        ):
            # ---- prefetch batch 0's x before the weights ----
            def emit_x_dma(b):
                xT_b = []
                for ct in range(CT):
                    t = xp.tile([P, NP], BF16, tag=f"x{ct}", name=f"x{ct}")
                    nc.sync.dma_start(
                        t[:], xT[ct * P : (ct + 1) * P, b * NP : (b + 1) * NP]
                    )
                    xT_b.append(t)
                return xT_b

            xT_b = emit_x_dma(0)

            # ---- PE warmup: junk matmuls overlap the input DMAs and get
            # the HAM clock gate to 8/8 before real work starts ----
            wup = singles.tile([P, 512], BF16, tag="wup")
            nc.vector.memset(wup[:], 1.0)
            for i in range(24):
                ps = pmm.tile([P, 512], F32, tag="pb", name="ps")
                nc.tensor.matmul(ps[:, :512], wup[:, 0:P], wup[:, 0:512],
                                 start=True, stop=True, skip_group_check=True)

            # ---- persistent loads (v/qk weights first) ----
            wv_sb = []
            wqk_sb = []
            wp_sb = []
            for ct in range(CT):
                t = singles.tile([P, C], BF16, tag=f"wv{ct}")
                nc.sync.dma_start(t[:], wvT[ct * P : (ct + 1) * P, :])
                wv_sb.append(t)
            for ct in range(CT):
                t = singles.tile([P, 2 * C], BF16, tag=f"wqk{ct}")
                nc.sync.dma_start(t[:], wqkT[ct * P : (ct + 1) * P, :])
                wqk_sb.append(t)
            bqk_sb = singles.tile([P, 2 * C // P], F32, tag="bqk")
            nc.sync.dma_start(bqk_sb[:], bqk.rearrange("(o p) -> p o", p=P))
            bv_sb = singles.tile([P, C], F32, tag="bv")
            nc.sync.dma_start(bv_sb[:], _bcast_ap(bv, P))
            ones_sb = singles.tile([P, H], BF16, tag="ones_sb")
            nc.sync.dma_start(ones_sb[:], _bcast_ap(ones12, P))
            mask0_sb = singles.tile([P, NP], BF16, tag="mask0")
            nc.sync.dma_start(mask0_sb[:], mask0d[:, :])
            maskw_sb = singles.tile([P, MW], BF16, tag="maskw")
            nc.sync.dma_start(maskw_sb[:], maskwd[:, :])
            for ct in range(CT):
                t = singles.tile([P, C], BF16, tag=f"wp{ct}")
                nc.sync.dma_start(t[:], wpT[ct * P : (ct + 1) * P, :])
                wp_sb.append(t)
            bp_sb = singles.tile([P, C], F32, tag="bp")
            nc.sync.dma_start(bp_sb[:], _bcast_ap(bp, P))

            def emit_v(xT_b):
                v_tok = []
                for mt, (m0, msz) in enumerate(KT):
                    vt = vtp.tile([P, H, D + 1], BF16, tag=f"vt{mt}", name=f"vt{mt}")
                    nc.vector.tensor_copy(vt[:, :, D : D + 1], ones_sb[:, :, None])
                    pool = pmm if mt % 2 == 0 else psc
                    tg = "pb" if mt % 2 == 0 else "sc"
                    pss = [
                        pool.tile([P, 512], F32, tag=tg, name="ps")
                        for _ in range(2)
                    ]
                    for ct in range(CT):
                        for ci, (c0, csz) in enumerate(VCH):
                            nc.tensor.matmul(
                                pss[ci][:msz, :csz],
                                xT_b[ct][:, m0 : m0 + msz],
                                wv_sb[ct][:, c0 : c0 + csz],
                                start=(ct == 0),
                                stop=(ct == CT - 1),
                            )
                    for ci, (c0, csz) in enumerate(VCH):
                        nh = csz // D
                        h0 = c0 // D
                        nc.vector.tensor_tensor(
                            vt[:msz, h0 : h0 + nh, 0:D],
                            pss[ci][:msz, :csz].rearrange("p (h d) -> p h d", d=D),
                            bv_sb[:msz, c0 : c0 + csz].rearrange(
                                "p (h d) -> p h d", d=D
                            ),
                            ALU.add,
                        )
                    v_tok.append(vt)
                return v_tok

            def emit_qk(hp, xT_b):
                qt = qkp.tile([P, NP], BF16, tag="qk_q")
                ktb = qkp.tile([P, NP], BF16, tag="qk_k")
                for dst, ft in ((qt, hp), (ktb, CT + hp)):
                    pss = [
                        pmm.tile([P, 512], F32, tag="pb", name="ps")
                        for _ in range(2)
                    ]
                    for ct in range(CT):
                        for ci, (c0, csz) in enumerate(QCH):
                            nc.tensor.matmul(
                                pss[ci][:, :csz],
                                wqk_sb[ct][:, ft * P : (ft + 1) * P],
                                xT_b[ct][:, c0 : c0 + csz],
                                start=(ct == 0),
                                stop=(ct == CT - 1),
                            )
                    for ci, (c0, csz) in enumerate(QCH):
                        nc.vector.scalar_tensor_tensor(
                            dst[:, c0 : c0 + csz],
                            pss[ci][:, :csz],
                            1.0,
                            bqk_sb[:, ft : ft + 1].to_broadcast([P, csz]),
                            ALU.mult,
                            ALU.add,
                        )
                return qt, ktb

            def emit_scores(qt, ktb):
                """scores -> exp -> mask for both heads of the pair.
                Returns es0[hi] (kt0, dense) and esg[hi][grp] (packed
                windows) bf16 tiles."""
                es0 = {}
                esg = {0: [None, None], 1: [None, None]}
                mm = nc.tensor.matmul
                for hi in (0, 1):
                    es0[hi] = esp.tile([P, 512], BF16, tag="es0", name="es0")
                # kt0 vs q [0, 512): fills one psum bank per head
                for hi in (0, 1):
                    po = D * hi
                    sc = psc.tile([P, 512], F32, tag="sc", name="sc")
                    mm(
                        sc[:, :512],
                        ktb[po : po + D, 0:128],
                        qt[po : po + D, 0:512],
                        start=True,
                        stop=True,
                    )
                    nc.scalar.activation(es0[hi][:, :512], sc[:, :512], AF.Exp)
                    eng = nc.vector if hi == 0 else nc.gpsimd
                    eng.tensor_tensor(
                        es0[hi][:, 0:512], es0[hi][:, 0:512], mask0_sb[:, 0:512],
                        ALU.mult,
                    )
                # kt1..4 banded windows (2 per psum tile) + kt0 vs q [512, 578)
                # packed into the grp-0 tile
                for grp in (0, 1):
                    wins = WIN[2 * grp : 2 * grp + 2]
                    for hi in (0, 1):
                        po = D * hi
                        sc = psc.tile([P, 512], F32, tag="sc", name="sc")
                        for g, off, a, blen, k0, ksz in wins:
                            mm(
                                sc[:ksz, off + 2 : off + 2 + blen],
                                ktb[po : po + D, k0 : k0 + ksz],
                                qt[po : po + D, a : a + blen],
                                start=True, stop=True, skip_group_check=True,
                            )
                            mm(
                                sc[:ksz, off : off + 2],
                                ktb[po : po + D, k0 : k0 + ksz],
                                qt[po : po + D, 0:2],
                                start=True, stop=True, skip_group_check=True,
                            )
                        if grp == 0:
                            mm(
                                sc[:, K0B : K0B + 66],
                                ktb[po : po + D, 0:128],
                                qt[po : po + D, 512:NP],
                                start=True, stop=True, skip_group_check=True,
                            )
                        gw = GW[grp]
                        es = eswp.tile([P, GW[0]], BF16, tag=f"esg{grp}",
                                       name=f"esg{grp}")
                        esg[hi][grp] = es
                        nc.scalar.activation(es[:, :gw], sc[:, :gw], AF.Exp)
                        eng = nc.vector if hi == 0 else nc.gpsimd
                        m0 = grp * GW[0]
                        eng.tensor_tensor(
                            es[:, :gw], es[:, :gw],
                            maskw_sb[:, m0 : m0 + gw], ALU.mult,
                        )
                return es0, esg

            def emit_av(hp, hi, es0, esg, v_tok, oc_sb, srs):
                """banded attn@v for head h, stage output + denominator."""
                h = 2 * hp + hi
                oe = poe.tile([D + 1, NP + 2], F32, tag="oe", name="oe")
                mm = nc.tensor.matmul
                # kt0 dense: first writer of both PSUM banks (start=True)
                mm(oe[:, 0:512], v_tok[0][:, h, :], es0[hi][:, 0:512],
                   start=True, stop=False, skip_group_check=True)
                mm(oe[:, 512:NP], v_tok[0][:, h, :],
                   esg[hi][0][:, K0B : K0B + 66],
                   start=True, stop=False, skip_group_check=True)
                # banded tiles: accumulate segments (split at bank boundary)
                for wi, (grp, off, a, blen, k0, ksz) in enumerate(WIN):
                    es = esg[hi][grp]
                    vkt = v_tok[wi + 1][:ksz, h, :]
                    s0 = off + 2
                    if a + blen <= 512:
                        segs = [(s0, s0 + blen, a)]
                    else:
                        sp = s0 + (512 - a)
                        segs = [(s0, sp, a), (sp, s0 + blen, 512)]
                    for g0, g1, o0 in segs:
                        mm(oe[:, o0 : o0 + (g1 - g0)], vkt, es[:ksz, g0:g1],
                           start=False, stop=False, skip_group_check=True)
                    # CLS query column accumulates into output column 0
                    # (column 1 adds masked zeros)
                    mm(oe[:, 0:2], vkt, es[:ksz, off : off + 2],
                       start=False, stop=(wi == len(WIN) - 1),
                       skip_group_check=True)
                # head output rows -> oc (hi=1 shifts partitions via DMA);
                # extract fp32 denominator row
                if hi == 0:
                    nc.vector.tensor_copy(oc_sb[hp][0:D, 0:N], oe[0:D, 0:N])
                else:
                    stage = rcpp.tile([D, NP], BF16, tag="stage")
                    nc.vector.tensor_copy(stage[:, 0:N], oe[0:D, 0:N])
                    nc.sync.dma_start(oc_sb[hp][D : 2 * D, 0:N], stage[:, 0:N])
                srf = rcpp.tile([D + 1, NP], F32, tag="srf")
                nc.scalar.copy(srf[D : D + 1, 0:N], oe[D : D + 1, 0:N])
                nc.sync.dma_start(srs[hi : hi + 1, 0:N], srf[D : D + 1, 0:N])

            def emit_recip(srs):
                """reciprocal of the pair's denominators + broadcast DMA."""
                rr = rcpp.tile([2, NP], F32, tag="rr")
                nc.vector.reciprocal_approx_fast(rr[:, 0:N], srs[:, 0:N])
                rrd = drp.tile([2, NP], F32, tag="rrd")
                nc.sync.dma_start(rrd[:, :], rr[:, :])
                rb = rcpp.tile([P, NP], F32, tag="rb")
                nc.sync.dma_start(rb[0:D, 0:N], _bcast_ap(rrd[0][0:N], D))
                nc.sync.dma_start(rb[D : 2 * D, 0:N], _bcast_ap(rrd[1][0:N], D))
                return rb

            def emit_nmul(hp, rb, oc_sb):
                oc = oc_sb[hp]
                for po in (0, D):
                    nc.vector.tensor_tensor(
                        oc[po : po + D, 0:N],
                        oc[po : po + D, 0:N],
                        rb[po : po + D, 0:N],
                        ALU.mult,
                    )

            def emit_proj(b, oc_sb):
                for mt, (m0, msz) in enumerate(KT):
                    ysb = ysp.tile([P, C], F32, tag="ysb", name="ysb")
                    pool = pmm if mt % 2 == 0 else psc
                    tg = "pb" if mt % 2 == 0 else "sc"
                    pss = [
                        pool.tile([P, 512], F32, tag=tg, name="ps")
                        for _ in range(2)
                    ]
                    for ct in range(CT):
                        for ci, (c0, csz) in enumerate(VCH):
                            nc.tensor.matmul(
                                pss[ci][:msz, :csz],
                                oc_sb[ct][:, m0 : m0 + msz],
                                wp_sb[ct][:, c0 : c0 + csz],
                                start=(ct == 0),
                                stop=(ct == CT - 1),
                            )
                    for ci, (c0, csz) in enumerate(VCH):
                        nc.vector.tensor_tensor(
                            ysb[:msz, c0 : c0 + csz],
                            pss[ci][:msz, :csz],
                            bp_sb[:msz, c0 : c0 + csz],
                            ALU.add,
                        )
                    nc.sync.dma_start(
                        y[b * N + m0 : b * N + m0 + msz, :], ysb[:msz, :]
                    )

            # ---- main schedule (software-pipelined across pairs/batches) ----
            v_tok = emit_v(xT_b)
            # cover the wqk DMA wait and warm the clock for the first qk
            for i in range(40):
                ps = pmm.tile([P, 512], F32, tag="pb", name="ps")
                nc.tensor.matmul(ps[:, :512], wup[:, 0:P], wup[:, 0:512],
                                 start=True, stop=True, skip_group_check=True)
            for b in range(BPC):
                oc_sb = [
                    ocp.tile([P, NP], BF16, tag=f"oc{ct}", name=f"oc{ct}")
                    for ct in range(CT)
                ]
                srs_l = []
                rb_l = []
                qk_t = emit_qk(0, xT_b)
                nxt_x = None
                for hp in range(H // 2):
                    qt, ktb = qk_t
                    es0, esg = emit_scores(qt, ktb)
                    if hp == 2 and b + 1 < BPC:
                        nxt_x = emit_x_dma(b + 1)
                    if hp + 1 < H // 2:
                        qk_t = emit_qk(hp + 1, xT_b)
                    srs = rcpp.tile([2, NP], F32, tag="srs")
                    srs_l.append(srs)
                    if hp >= 1:
                        rb_l.append(emit_recip(srs_l[hp - 1]))
                    for hi in (0, 1):
                        emit_av(hp, hi, es0, esg, v_tok, oc_sb, srs)
                    if hp >= 1:
                        emit_nmul(hp - 1, rb_l[hp - 1], oc_sb)
                prev_oc = oc_sb
                rb = emit_recip(srs_l[H // 2 - 1])
                if b + 1 < BPC:
                    xT_b = nxt_x
                    v_tok = emit_v(xT_b)
                else:
                    # no next-batch v GEMM to cover the final norm chain:
                    # keep the PE busy/warm with junk matmuls instead
                    for i in range(12):
                        ps = pmm.tile([P, 512], F32, tag="pb", name="ps")
                        nc.tensor.matmul(ps[:, :512], wup[:, 0:P], wup[:, 0:512],
                                         start=True, stop=True,
                                         skip_group_check=True)
                emit_nmul(H // 2 - 1, rb, prev_oc)
                emit_proj(b, prev_oc)

    nc.finalize()
    return nc


_CACHE = {}


def _make_in_maps(x, qkv_w, qkv_b, proj_w, proj_b):
    bf = ml_dtypes.bfloat16
    x = np.asarray(x, np.float32)
    qkv_w = np.asarray(qkv_w, np.float32)
    qkv_b = np.asarray(qkv_b, np.float32)
    proj_w = np.asarray(proj_w, np.float32)
    proj_b = np.asarray(proj_b, np.float32)

    wqkT = np.ascontiguousarray(qkv_w[: 2 * C].T).copy()
    wqkT[:, :C] *= SCALE
    wqkT = wqkT.astype(bf)
    wvT = np.ascontiguousarray(qkv_w[2 * C :].T).astype(bf)
    wpT = np.ascontiguousarray(proj_w.T).astype(bf)
    bqk_h = qkv_b[: 2 * C].copy()
    bqk_h[:C] *= SCALE
    bv_h = np.ascontiguousarray(qkv_b[2 * C :])

    m = np.zeros((NP, NP), np.float32)
    m[:N, :N] = _build_mask_np()
    mask0 = m[:P, :].astype(bf)
    maskw = np.zeros((P, MW), np.float32)
    for grp, off, a, blen, k0, ksz in WIN:
        base = grp * GW[0] + off
        maskw[:ksz, base] = 1.0
        maskw[:ksz, base + 1] = m[k0 : k0 + ksz, 1]
        maskw[:ksz, base + 2 : base + 2 + blen] = m[k0 : k0 + ksz, a : a + blen]
    maskw[:, K0B : K0B + 66] = m[:P, 512:NP]
    maskw = maskw.astype(bf)

    in_maps = []
    for c in range(NCORES):
        xp_c = np.zeros((BPC, NP, C), np.float32)
        xp_c[:, :N, :] = x[c * BPC : (c + 1) * BPC]
        xT_c = np.ascontiguousarray(xp_c.reshape(TP, C).T).astype(bf)
        in_maps.append(
            {
                "xT": xT_c,
                "wqkT": wqkT,
                "wvT": wvT,
                "wpT": wpT,
                "bqk": bqk_h,
                "bv": bv_h,
                "bp": proj_b,
                "mask0d": mask0,
                "maskwd": maskw,
                "ones12": np.ones(H, bf),
            }
        )
    return in_maps


def kernel(x, qkv_w, qkv_b, proj_w, proj_b):
    if "nc" not in _CACHE:
        _CACHE["nc"] = _build_program()
    nc = _CACHE["nc"]

    in_maps = _make_in_maps(x, qkv_w, qkv_b, proj_w, proj_b)
    res = run_bass_kernel_spmd(nc, in_maps, list(range(NCORES)))
    out = np.concatenate(
        [res.results[c]["y"].reshape(BPC, N, C) for c in range(NCORES)], axis=0
    )
    return out.astype(np.float32)
